# revision 56
# baseline (speedup 1.0000x reference)
"""EvolveGCN-O forward pass on 8 Trainium2 NeuronCores (Bass/Tile).

Math (reference):
    w_new = LSTM-evolve(weight; w_ih, b_ih+b_hh)          # [C, C]
    out   = D^-1/2 (A + I) D^-1/2  X  w_new               # [N, C]

Device strategy (per sharding hint: edges + scatter targets sharded):
  * Destination nodes padded to NPAD (multiple of 128*8); 128-node
    blocks; each core owns nbc consecutive blocks, processed in chunks
    of 7 (7 PSUM banks accumulate 7 blocks; the 8th bank holds Y).
  * Self-loop term dinv[i]^2 x[i]: contiguous x rows loaded directly,
    scaled on the scalar engine, transposed into the block's PSUM
    accumulator via an identity matmul (start of each accumulation).
  * Edges: host sorts by dst block and splits by source range (the
    dma_gather index is a SIGNED int16 offset from the call's base row,
    so one call reaches a 65536-row window -> 2 ranges cover N=100k).
    Every (block, range) segment is padded to a uniform tile count
    (norm=0 padding, >=1 slack slot so no call ends on a negative
    index, which the ucode would drop).
  * Per edge tile of 128: gpsimd.dma_gather stages rows x[src] (one
    call per (chunk, range) section, ~4-6k rows); scalar engine scales
    by the per-edge norm dinv[src]*dinv[dst]; vector engine builds the
    one-hot dst selector via is_equal(dstl, iota); PE accumulates
    aggT += M^T @ S. Per block: Y = aggT^T @ w_new, DMA out.
  * w_new computed on-device (3 matmuls + activations), redundantly per
    core. No collectives: block ownership makes outputs disjoint.
"""
import sys

for _p in ("/opt/trn_rl_repo", "/root/.axon_site/_ro/trn_rl_repo"):
    if _p not in sys.path:
        sys.path.append(_p)

import ml_dtypes
import numpy as np

BF16 = ml_dtypes.bfloat16

N, C, E = 100000, 128, 1600000  # problem shape (hardcoded per spec)
P = 128
N_CORES = 8
CHUNK = 7  # blocks per PSUM-resident chunk (7 psA banks + psB = 8)
IDX_WIN = 32768  # int16 signed reach below/above base
import os as _os

CALL_T = int(_os.environ.get("CALL_T", "8"))  # edge tiles per dma_gather call
NQ = 4  # SWDGE queues: gather desc-gen runs on Q7 core pair 2q/2q+1


def _cdiv(a, b):
    return -(-a // b)


def prep_inputs(x, edge_index, weight, w_ih, b_ih, b_hh, n=N):
    """Host-side sharding/index prep.

    Returns (in_maps, meta) where meta = (t_r tuple, nbc, chunk_sizes).
    """
    x = np.ascontiguousarray(np.asarray(x, dtype=np.float32))
    ei = np.asarray(edge_index)
    src_e = ei[0].astype(np.int64)
    dst_e = ei[1].astype(np.int64)

    npad = _cdiv(n, P * N_CORES) * P * N_CORES
    nb = npad // P
    nbc = nb // N_CORES

    # degrees include self loops
    deg = (np.bincount(dst_e, minlength=n) + 1).astype(np.float32)
    dinv = (1.0 / np.sqrt(deg)).astype(np.float32)
    d2 = np.zeros(npad, np.float32)  # dinv[dst], applied at the flush
    d2[:n] = dinv

    # dinv[src] folded into the gathered rows host-side
    xpad = np.zeros((npad, C), BF16)
    xpad[:n] = (x * dinv[:, None]).astype(BF16)

    # source ranges
    split = npad // 2
    bases = (max(0, split - IDX_WIN), max(0, npad - IDX_WIN))
    los = (0, split)
    his = (split, npad)
    rng_of = (src_e >= split).astype(np.int64)

    # sort edges by (block, range) then pack
    blk = dst_e >> 7
    order = np.argsort(blk * 2 + rng_of, kind="stable")
    srcs = src_e[order]
    dsts = dst_e[order]
    rngs = rng_of[order]
    blks = blk[order]

    # per-(block, range) counts -> uniform tile counts. +4 slack slots so
    # every cell keeps spare padding for the call-final swap below.
    cell = blks * 2 + rngs
    counts = np.bincount(cell, minlength=nb * 2).reshape(nb, 2)
    t_r = tuple(int(_cdiv(int(counts[:, r].max()) + 4, P)) for r in range(2))
    cap = (t_r[0] * P, t_r[1] * P)

    # chunk structure (uniform across cores)
    chunk_sizes = [min(CHUNK, nbc - i) for i in range(0, nbc, CHUNK)]

    # pack edges into per-(block, range) padded slots
    cell_cap = np.array([cap[0], cap[1]], np.int64)
    cell_starts = np.zeros(nb * 2 + 1, np.int64)
    np.cumsum(counts.reshape(-1), out=cell_starts[1:])
    pos_in_cell = np.arange(len(srcs)) - cell_starts[cell]
    slot = cell * 0  # placeholder
    # flat slot index: block-major [b][r][slot]
    cell_base = np.zeros(nb * 2, np.int64)
    cell_base[0::2] = np.arange(nb) * (cap[0] + cap[1])
    cell_base[1::2] = cell_base[0::2] + cap[0]
    flat = cell_base[cell] + pos_in_cell

    tot = nb * (cap[0] + cap[1])
    idx_all = np.zeros(tot, np.int32)  # padding idx = 0 (valid row at base)
    # padding dstl = 200: matches no iota lane -> one-hot column all-zero
    dstl_all = np.full(tot, 200.0, np.float32)
    idx_all[flat] = (srcs - np.array(bases)[rngs]).astype(np.int32)
    dstl_all[flat] = (dsts & (P - 1)).astype(np.float32)


    wt = np.ascontiguousarray(np.asarray(weight, np.float32).T)
    wiht = np.ascontiguousarray(np.asarray(w_ih, np.float32).T)
    bsum = (
        (np.asarray(b_ih, np.float32) + np.asarray(b_hh, np.float32))
        .reshape(4, C)
        .T.copy()
    )

    per_blk = cap[0] + cap[1]
    in_maps = []
    for m in range(N_CORES):
        lo_b = m * nbc
        seg = slice(lo_b * per_blk, (lo_b + nbc) * per_blk)
        idx_c = idx_all[seg].reshape(nbc, per_blk)
        dstl_c = dstl_all[seg].reshape(nbc, per_blk)

        # build per-(chunk, range) sections: [sections] each a flat idx list
        gidx_secs = []
        gdstl_secs = []
        b0 = 0
        for cs in chunk_sizes:
            for r in range(2):
                off = 0 if r == 0 else cap[0]
                sec_idx = idx_c[b0 : b0 + cs, off : off + cap[r]].reshape(-1).copy()
                sec_dstl = (
                    dstl_c[b0 : b0 + cs, off : off + cap[r]].reshape(-1).copy()
                )
                # the gather ucode DROPS a trailing negative index, so the
                # final slot of every call must be >= 0: swap offending real
                # edges with a padding slot of the SAME (block,range) cell.
                sec_tiles = cs * t_r[r]
                call_ts = [CALL_T] * (sec_tiles // CALL_T)
                if sec_tiles % CALL_T:
                    call_ts.append(sec_tiles % CALL_T)
                ends = np.cumsum(np.array(call_ts)) * P - 1  # call-final slots
                end_set = set(int(e) for e in ends)
                for s in ends:
                    s = int(s)
                    if sec_idx[s] >= 0:
                        continue
                    k = s // cap[r]  # cell (block) within section
                    cnt = int(counts[lo_b + b0 + k, r])
                    for p in range(k * cap[r] + cnt, (k + 1) * cap[r]):
                        if p not in end_set:
                            for arr in (sec_idx, sec_dstl):
                                arr[s], arr[p] = arr[p], arr[s]
                            break
                    else:
                        raise RuntimeError("no swap slot for call-final pad")
                gidx_secs.append(sec_idx)
                gdstl_secs.append(sec_dstl)
            b0 += cs

        # idx wrap per CALL: i -> [i%16, i//16], replicated x8 across
        # partitions; concatenate calls/sections along free dim
        def wrap16(a):
            w = np.ascontiguousarray(a.reshape(-1, 16).T.astype(np.int16))
            return np.tile(w, (8, 1))  # [128, len/16]

        gidx_cols = []
        for sec in gidx_secs:
            st = len(sec) // P
            for c0 in range(0, st, CALL_T):
                ct = min(CALL_T, st - c0)
                gidx_cols.append(wrap16(sec[c0 * P : (c0 + ct) * P]))
        gidx = np.concatenate(gidx_cols, axis=1)

        # per-tile metadata, partition-major per section: [128, tiles]
        def tilemeta(secs):
            cols = [s.reshape(-1, P).T for s in secs]  # [128, tiles_sec]
            return np.ascontiguousarray(np.concatenate(cols, axis=1))

        # one-hot scatter tiles precomputed host-side, streamed via HWDGE:
        # oh[p, j*128+l] = 1 iff tile j's slot p targets lane l (pad -> all 0)
        gdstl = tilemeta(gdstl_secs)  # [128, n_tiles] fp32
        gsoh = (
            (gdstl[:, :, None] == np.arange(P, dtype=np.float32)[None, None, :])
            .astype(BF16)
            .reshape(P, -1)
        )

        d2_c = d2[lo_b * P : (lo_b + nbc) * P].reshape(nbc, P).T.copy()  # [128, nbc]

        in_maps.append(
            dict(
                xsrc=xpad,
                xself=np.ascontiguousarray(xpad[lo_b * P : (lo_b + nbc) * P]),
                gidx=np.ascontiguousarray(gidx),
                gsoh=np.ascontiguousarray(gsoh),
                d2=d2_c,
                wt=wt,
                wiht=wiht,
                bias=bsum,
            )
        )
    meta = (t_r, nbc, chunk_sizes, bases, npad)
    return in_maps, meta


def build_program(meta, reps=1, variant="full", nq=NQ):
    import contextlib

    import concourse.bacc as bacc
    import concourse.mybir as mybir
    import concourse.tile as tile
    from concourse.masks import make_identity

    t_r, nbc, chunk_sizes, bases, npad = meta
    f32 = mybir.dt.float32
    bf16 = mybir.dt.bfloat16
    i16 = mybir.dt.int16
    AF = mybir.ActivationFunctionType
    ALU = mybir.AluOpType

    n_tiles = nbc * (t_r[0] + t_r[1])  # edge tiles per core
    idx_w = n_tiles * P // 16  # gidx free dim
    nsec = 2 * len(chunk_sizes)

    nc = bacc.Bacc("TRN2", num_swdge_queues=nq)
    xsrc = nc.declare_dram_parameter("xsrc", [npad, C], bf16, isOutput=False)
    xself = nc.declare_dram_parameter("xself", [nbc * P, C], bf16, isOutput=False)
    gidx = nc.declare_dram_parameter("gidx", [P, idx_w], i16, isOutput=False)
    gsoh = nc.declare_dram_parameter("gsoh", [P, n_tiles * P], bf16, isOutput=False)
    d2 = nc.declare_dram_parameter("d2", [P, nbc], f32, isOutput=False)
    wt = nc.declare_dram_parameter("wt", [P, P], f32, isOutput=False)
    wiht = nc.declare_dram_parameter("wiht", [P, 4 * C], f32, isOutput=False)
    bias = nc.declare_dram_parameter("bias", [P, 4], f32, isOutput=False)
    out = nc.declare_dram_parameter("out", [nbc * P, C], f32, isOutput=True)

    with tile.TileContext(nc) as tc:
        with (
            tc.tile_pool(name="const", bufs=1) as constp,
            tc.tile_pool(name="stag", bufs=3) as stagp,
            tc.tile_pool(name="meta", bufs=2) as metap,
            tc.tile_pool(name="work", bufs=2) as workp,
            tc.tile_pool(name="selfx", bufs=3) as selfp,
            tc.tile_pool(name="psA", bufs=CHUNK, space="PSUM") as psA,
            tc.tile_pool(name="psB", bufs=1, space="PSUM") as psB,
            tc.tile_pool(name="osb", bufs=3) as osbp,
        ):

            wt_sb = constp.tile([P, P], f32, tag="wt")
            nc.sync.dma_start(out=wt_sb[:], in_=wt[:])
            wiht_sb = constp.tile([P, 4 * C], f32, tag="wiht")
            nc.sync.dma_start(out=wiht_sb[:], in_=wiht[:])
            bias_sb = constp.tile([P, 4], f32, tag="bias")
            nc.sync.dma_start(out=bias_sb[:], in_=bias[:])
            d2_sb = constp.tile([P, nbc], f32, tag="d2")
            nc.sync.dma_start(out=d2_sb[:], in_=d2[:])
            ident = constp.tile([P, P], f32, tag="ident")
            make_identity(nc, ident[:])
            identb = constp.tile([P, P], bf16, tag="identb")
            make_identity(nc, identb[:])

            # --- LSTM single step -> evolved weight w_new ---
            gate_sb = {}
            for m, func, bcol in ((0, AF.Sigmoid, 0), (2, AF.Tanh, 2), (3, AF.Sigmoid, 3)):
                ps = psB.tile([P, P], f32, tag="psb")
                nc.tensor.matmul(
                    out=ps[:],
                    lhsT=wiht_sb[:, m * P : (m + 1) * P],
                    rhs=wt_sb[:],
                    start=True,
                    stop=True,
                )
                sb = constp.tile([P, P], f32, tag=f"gate{m}")
                nc.scalar.activation(
                    out=sb[:], in_=ps[:], func=func, bias=bias_sb[:, bcol : bcol + 1]
                )
                gate_sb[m] = sb
            cT = constp.tile([P, P], f32, tag="cT")
            nc.vector.tensor_mul(out=cT[:], in0=gate_sb[0][:], in1=gate_sb[2][:])
            tcT = constp.tile([P, P], f32, tag="tcT")
            nc.scalar.activation(out=tcT[:], in_=cT[:], func=AF.Tanh)
            wnT = constp.tile([P, P], f32, tag="wnT")
            nc.vector.tensor_mul(out=wnT[:], in0=gate_sb[3][:], in1=tcT[:])
            wn_ps = psB.tile([P, P], f32, tag="psb")
            nc.tensor.transpose(out=wn_ps[:], in_=wnT[:], identity=ident[:])
            wn_sb = constp.tile([P, P], bf16, tag="wn")
            nc.vector.tensor_copy(out=wn_sb[:], in_=wn_ps[:])

            # --- main: chunks of blocks; self pass + 2 gather passes ---
            do_pe = variant in ("full", "noscat", "deadsoh")
            do_dve = variant in ("full", "gatherdve", "deadsoh")
            use_soh = variant == "full"
            do_gather = variant != "nogather"

            def emit_main(_iv=None):
              b0 = 0  # first block of chunk
              s_tile = 0  # global edge-tile cursor
              s_idx = 0  # global gidx column cursor (int16 cols)
              n_call = 0  # gather call counter (queue round-robin)
              for cs in chunk_sizes:
                if do_pe:
                    aggs = [
                        psA.tile([P, P], f32, name=f"agg{i}", tag="agg")
                        for i in range(cs)
                    ]
                    # self-loop pass: aggT[b] = (dinv*x_block)^T; the second
                    # dinv factor is applied at the flush
                    for i in range(cs):
                        b = b0 + i
                        xs = selfp.tile([P, P], bf16, tag="xself")
                        nc.sync.dma_start(
                            out=xs[:], in_=xself[b * P : (b + 1) * P, :]
                        )
                        nc.tensor.matmul(
                            out=aggs[i][:], lhsT=xs[:], rhs=identb[:],
                            start=True, stop=False,
                        )
                # gather passes
                for r in range(2):
                    sec_tiles = cs * t_r[r]
                    num = sec_tiles * P
                    idx_t = metap.tile([P, sec_tiles * 8], i16, tag="idx")
                    nc.sync.dma_start(
                        out=idx_t[:], in_=gidx[:, s_idx : s_idx + sec_tiles * 8]
                    )
                    stag = stagp.tile([P, sec_tiles * P], bf16, tag="stag")
                    for c0 in range(0, sec_tiles, CALL_T):
                        if not do_gather:
                            break
                        ct = min(CALL_T, sec_tiles - c0)
                        nc.gpsimd.dma_gather(
                            out_ap=stag[:, c0 * P : (c0 + ct) * P].rearrange(
                                "p (t c) -> p t c", t=ct
                            ),
                            in_ap=xsrc[bases[r] :, :],
                            idxs_ap=idx_t[:, c0 * 8 : (c0 + ct) * 8],
                            num_idxs=ct * P,
                            num_idxs_reg=ct * P,
                            elem_size=P,
                            queue_num=n_call % nq,
                        )
                        n_call += 1
                    if not (do_pe or do_dve):
                        s_tile += sec_tiles
                        s_idx += sec_tiles * 8
                        continue
                    if do_dve:
                        # host-precomputed one-hot scatter tiles, streamed via
                        # HWDGE (no Q7 work, no DVE: both would starve the
                        # SWDGE gather descriptor generation)
                        s_oh = workp.tile([P, sec_tiles * P], bf16, tag="soh")
                        nc.sync.dma_start(
                            out=s_oh[:],
                            in_=gsoh[:, s_tile * P : (s_tile + sec_tiles) * P],
                        )
                    for i in range(cs):
                        last_of_block = (r == 1)
                        for t in range(t_r[r]):
                            j = i * t_r[r] + t  # tile within section
                            if do_pe:
                                nc.tensor.matmul(
                                    out=aggs[i][:],
                                    lhsT=stag[:, j * P : (j + 1) * P],
                                    rhs=s_oh[:, j * P : (j + 1) * P]
                                    if use_soh
                                    else identb[:],
                                    start=False,
                                    stop=(last_of_block and t == t_r[r] - 1),
                                )
                    s_tile += sec_tiles
                    s_idx += sec_tiles * 8
                # flush chunk
                for i in range(cs if do_pe else 0):
                    b = b0 + i
                    agg_sb = osbp.tile([P, P], bf16, tag="aggsb")
                    nc.scalar.activation(out=agg_sb[:], in_=aggs[i][:], func=AF.Copy)
                    y_ps = psB.tile([P, P], f32, tag="psb")
                    nc.tensor.matmul(
                        out=y_ps[:], lhsT=agg_sb[:], rhs=wn_sb[:],
                        start=True, stop=True,
                    )
                    y_sb = osbp.tile([P, P], f32, tag="ysb")
                    nc.scalar.activation(
                        out=y_sb[:], in_=y_ps[:], func=AF.Copy,
                        scale=d2_sb[:, b : b + 1],
                    )
                    nc.sync.dma_start(
                        out=out[b * P : (b + 1) * P, :], in_=y_sb[:]
                    )
                b0 += cs

            if reps > 1:
                with tc.For_i(0, reps, 1):
                    emit_main()
            else:
                emit_main()

    nc.finalize()
    return nc


def kernel(**inputs) -> np.ndarray:
    from concourse.bass_utils import run_bass_kernel_spmd

    x = inputs["x"]
    n = x.shape[0]
    in_maps, meta = prep_inputs(
        x,
        inputs["edge_index"],
        inputs["weight"],
        inputs["w_ih"],
        inputs["b_ih"],
        inputs["b_hh"],
        n=n,
    )
    nc = build_program(meta)
    res = run_bass_kernel_spmd(nc, in_maps, list(range(N_CORES)))
    full = np.concatenate([r["out"] for r in res.results], axis=0)
    return np.ascontiguousarray(full[:n])



# revision 58
# speedup vs baseline: 1.0966x; 1.0966x over previous
"""EvolveGCN-O forward pass on 8 Trainium2 NeuronCores (Bass/Tile).

Math (reference):
    w_new = LSTM-evolve(weight; w_ih, b_ih+b_hh)          # [C, C]
    out   = D^-1/2 (A + I) D^-1/2  X  w_new               # [N, C]

Device strategy (per sharding hint: edges + scatter targets sharded):
  * Destination nodes padded to NPAD (multiple of 128*8); 128-node
    blocks; each core owns nbc consecutive blocks, processed in chunks
    of 7 (7 PSUM banks accumulate 7 blocks; the 8th bank holds Y).
  * Self-loop term dinv[i]^2 x[i]: contiguous x rows loaded directly,
    scaled on the scalar engine, transposed into the block's PSUM
    accumulator via an identity matmul (start of each accumulation).
  * Edges: host sorts by dst block and splits by source range (the
    dma_gather index is a SIGNED int16 offset from the call's base row,
    so one call reaches a 65536-row window -> 2 ranges cover N=100k).
    Every (block, range) segment is padded to a uniform tile count
    (norm=0 padding, >=1 slack slot so no call ends on a negative
    index, which the ucode would drop).
  * Per edge tile of 128: gpsimd.dma_gather stages rows x[src] (one
    call per (chunk, range) section, ~4-6k rows); scalar engine scales
    by the per-edge norm dinv[src]*dinv[dst]; vector engine builds the
    one-hot dst selector via is_equal(dstl, iota); PE accumulates
    aggT += M^T @ S. Per block: Y = aggT^T @ w_new, DMA out.
  * w_new computed on-device (3 matmuls + activations), redundantly per
    core. No collectives: block ownership makes outputs disjoint.
"""
import sys

for _p in ("/opt/trn_rl_repo", "/root/.axon_site/_ro/trn_rl_repo"):
    if _p not in sys.path:
        sys.path.append(_p)

import ml_dtypes
import numpy as np

BF16 = ml_dtypes.bfloat16

N, C, E = 100000, 128, 1600000  # problem shape (hardcoded per spec)
P = 128
N_CORES = 8
CHUNK = 7  # blocks per PSUM-resident chunk (7 psA banks + psB = 8)
IDX_WIN = 32768  # int16 signed reach below/above base
import os as _os

CALL_T = int(_os.environ.get("CALL_T", "8"))  # edge tiles per dma_gather call
NQ = 4  # SWDGE queues: gather desc-gen runs on Q7 core pair 2q/2q+1


def _cdiv(a, b):
    return -(-a // b)


def prep_inputs(x, edge_index, weight, w_ih, b_ih, b_hh, n=N):
    """Host-side sharding/index prep.

    Returns (in_maps, meta) where meta = (t_r tuple, nbc, chunk_sizes).
    """
    x = np.ascontiguousarray(np.asarray(x, dtype=np.float32))
    ei = np.asarray(edge_index)
    src_e = ei[0].astype(np.int64)
    dst_e = ei[1].astype(np.int64)

    npad = _cdiv(n, P * N_CORES) * P * N_CORES
    nb = npad // P
    nbc = nb // N_CORES

    # degrees include self loops
    deg = (np.bincount(dst_e, minlength=n) + 1).astype(np.float32)
    dinv = (1.0 / np.sqrt(deg)).astype(np.float32)
    d2 = np.zeros(npad, np.float32)  # dinv[dst], applied at the flush
    d2[:n] = dinv

    # dinv[src] folded into the gathered rows host-side
    xpad = np.zeros((npad, C), BF16)
    xpad[:n] = (x * dinv[:, None]).astype(BF16)

    # source ranges
    split = npad // 2
    bases = (max(0, split - IDX_WIN), max(0, npad - IDX_WIN))
    los = (0, split)
    his = (split, npad)
    rng_of = (src_e >= split).astype(np.int64)

    # sort edges by (block, range) then pack
    blk = dst_e >> 7
    order = np.argsort(blk * 2 + rng_of, kind="stable")
    srcs = src_e[order]
    dsts = dst_e[order]
    rngs = rng_of[order]
    blks = blk[order]

    # per-(block, range) counts -> uniform tile counts. +4 slack slots so
    # every cell keeps spare padding for the call-final swap below.
    cell = blks * 2 + rngs
    counts = np.bincount(cell, minlength=nb * 2).reshape(nb, 2)
    t_r = tuple(int(_cdiv(int(counts[:, r].max()) + 4, P)) for r in range(2))
    cap = (t_r[0] * P, t_r[1] * P)

    # chunk structure (uniform across cores)
    chunk_sizes = [min(CHUNK, nbc - i) for i in range(0, nbc, CHUNK)]

    # pack edges into per-(block, range) padded slots
    cell_cap = np.array([cap[0], cap[1]], np.int64)
    cell_starts = np.zeros(nb * 2 + 1, np.int64)
    np.cumsum(counts.reshape(-1), out=cell_starts[1:])
    pos_in_cell = np.arange(len(srcs)) - cell_starts[cell]
    slot = cell * 0  # placeholder
    # flat slot index: block-major [b][r][slot]
    cell_base = np.zeros(nb * 2, np.int64)
    cell_base[0::2] = np.arange(nb) * (cap[0] + cap[1])
    cell_base[1::2] = cell_base[0::2] + cap[0]
    flat = cell_base[cell] + pos_in_cell

    tot = nb * (cap[0] + cap[1])
    idx_all = np.zeros(tot, np.int32)  # padding idx = 0 (valid row at base)
    # padding dstl = 200: matches no iota lane -> one-hot column all-zero
    dstl_all = np.full(tot, 200.0, np.float32)
    idx_all[flat] = (srcs - np.array(bases)[rngs]).astype(np.int32)
    dstl_all[flat] = (dsts & (P - 1)).astype(np.float32)


    wt = np.ascontiguousarray(np.asarray(weight, np.float32).T)
    wiht = np.ascontiguousarray(np.asarray(w_ih, np.float32).T)
    bsum = (
        (np.asarray(b_ih, np.float32) + np.asarray(b_hh, np.float32))
        .reshape(4, C)
        .T.copy()
    )

    per_blk = cap[0] + cap[1]
    in_maps = []
    for m in range(N_CORES):
        lo_b = m * nbc
        seg = slice(lo_b * per_blk, (lo_b + nbc) * per_blk)
        idx_c = idx_all[seg].reshape(nbc, per_blk)
        dstl_c = dstl_all[seg].reshape(nbc, per_blk)

        # build per-(chunk, range) sections: [sections] each a flat idx list
        gidx_secs = []
        gdstl_secs = []
        b0 = 0
        for cs in chunk_sizes:
            for r in range(2):
                off = 0 if r == 0 else cap[0]
                sec_idx = idx_c[b0 : b0 + cs, off : off + cap[r]].reshape(-1).copy()
                sec_dstl = (
                    dstl_c[b0 : b0 + cs, off : off + cap[r]].reshape(-1).copy()
                )
                # the gather ucode DROPS a trailing negative index, so the
                # final slot of every call must be >= 0: swap offending real
                # edges with a padding slot of the SAME (block,range) cell.
                sec_tiles = cs * t_r[r]
                call_ts = [CALL_T] * (sec_tiles // CALL_T)
                if sec_tiles % CALL_T:
                    call_ts.append(sec_tiles % CALL_T)
                ends = np.cumsum(np.array(call_ts)) * P - 1  # call-final slots
                end_set = set(int(e) for e in ends)
                for s in ends:
                    s = int(s)
                    if sec_idx[s] >= 0:
                        continue
                    k = s // cap[r]  # cell (block) within section
                    cnt = int(counts[lo_b + b0 + k, r])
                    for p in range(k * cap[r] + cnt, (k + 1) * cap[r]):
                        if p not in end_set:
                            for arr in (sec_idx, sec_dstl):
                                arr[s], arr[p] = arr[p], arr[s]
                            break
                    else:
                        raise RuntimeError("no swap slot for call-final pad")
                gidx_secs.append(sec_idx)
                gdstl_secs.append(sec_dstl)
            b0 += cs

        # idx wrap per CALL: i -> [i%16, i//16], replicated x8 across
        # partitions; concatenate calls/sections along free dim
        def wrap16(a):
            w = np.ascontiguousarray(a.reshape(-1, 16).T.astype(np.int16))
            return np.tile(w, (8, 1))  # [128, len/16]

        gidx_cols = []
        for sec in gidx_secs:
            st = len(sec) // P
            for c0 in range(0, st, CALL_T):
                ct = min(CALL_T, st - c0)
                gidx_cols.append(wrap16(sec[c0 * P : (c0 + ct) * P]))
        gidx = np.concatenate(gidx_cols, axis=1)

        # per-tile metadata, partition-major per section: [128, tiles]
        def tilemeta(secs):
            cols = [s.reshape(-1, P).T for s in secs]  # [128, tiles_sec]
            return np.ascontiguousarray(np.concatenate(cols, axis=1))

        # one-hot scatter tiles precomputed host-side, streamed via HWDGE:
        # oh[p, j*128+l] = 1 iff tile j's slot p targets lane l (pad -> all 0)
        gdstl = tilemeta(gdstl_secs)  # [128, n_tiles] fp32
        gsoh = (
            (gdstl[:, :, None] == np.arange(P, dtype=np.float32)[None, None, :])
            .astype(ml_dtypes.float8_e4m3)
            .reshape(P, -1)
        )

        d2_c = d2[lo_b * P : (lo_b + nbc) * P].reshape(nbc, P).T.copy()  # [128, nbc]

        in_maps.append(
            dict(
                xsrc=xpad,
                xself=np.ascontiguousarray(xpad[lo_b * P : (lo_b + nbc) * P]),
                gidx=np.ascontiguousarray(gidx),
                gsoh=np.ascontiguousarray(gsoh),
                d2=d2_c,
                wt=wt,
                wiht=wiht,
                bias=bsum,
            )
        )
    meta = (t_r, nbc, chunk_sizes, bases, npad)
    return in_maps, meta


def build_program(meta, reps=1, variant="full", nq=NQ):
    import contextlib

    import concourse.bacc as bacc
    import concourse.mybir as mybir
    import concourse.tile as tile
    from concourse.masks import make_identity

    t_r, nbc, chunk_sizes, bases, npad = meta
    f32 = mybir.dt.float32
    bf16 = mybir.dt.bfloat16
    i16 = mybir.dt.int16
    AF = mybir.ActivationFunctionType
    ALU = mybir.AluOpType

    n_tiles = nbc * (t_r[0] + t_r[1])  # edge tiles per core
    idx_w = n_tiles * P // 16  # gidx free dim
    nsec = 2 * len(chunk_sizes)

    nc = bacc.Bacc("TRN2", num_swdge_queues=nq)
    xsrc = nc.declare_dram_parameter("xsrc", [npad, C], bf16, isOutput=False)
    xself = nc.declare_dram_parameter("xself", [nbc * P, C], bf16, isOutput=False)
    gidx = nc.declare_dram_parameter("gidx", [P, idx_w], i16, isOutput=False)
    gsoh = nc.declare_dram_parameter("gsoh", [P, n_tiles * P], mybir.dt.float8e4, isOutput=False)
    d2 = nc.declare_dram_parameter("d2", [P, nbc], f32, isOutput=False)
    wt = nc.declare_dram_parameter("wt", [P, P], f32, isOutput=False)
    wiht = nc.declare_dram_parameter("wiht", [P, 4 * C], f32, isOutput=False)
    bias = nc.declare_dram_parameter("bias", [P, 4], f32, isOutput=False)
    out = nc.declare_dram_parameter("out", [nbc * P, C], f32, isOutput=True)

    with tile.TileContext(nc) as tc:
        with (
            tc.tile_pool(name="const", bufs=1) as constp,
            tc.tile_pool(name="stag", bufs=3) as stagp,
            tc.tile_pool(name="meta", bufs=2) as metap,
            tc.tile_pool(name="work", bufs=2) as workp,
            tc.tile_pool(name="selfx", bufs=3) as selfp,
            tc.tile_pool(name="psA", bufs=CHUNK, space="PSUM") as psA,
            tc.tile_pool(name="psB", bufs=1, space="PSUM") as psB,
            tc.tile_pool(name="osb", bufs=3) as osbp,
        ):

            wt_sb = constp.tile([P, P], f32, tag="wt")
            nc.sync.dma_start(out=wt_sb[:], in_=wt[:])
            wiht_sb = constp.tile([P, 4 * C], f32, tag="wiht")
            nc.sync.dma_start(out=wiht_sb[:], in_=wiht[:])
            bias_sb = constp.tile([P, 4], f32, tag="bias")
            nc.sync.dma_start(out=bias_sb[:], in_=bias[:])
            d2_sb = constp.tile([P, nbc], f32, tag="d2")
            nc.sync.dma_start(out=d2_sb[:], in_=d2[:])
            ident = constp.tile([P, P], f32, tag="ident")
            make_identity(nc, ident[:])
            identb = constp.tile([P, P], bf16, tag="identb")
            make_identity(nc, identb[:])

            # --- LSTM single step -> evolved weight w_new ---
            gate_sb = {}
            for m, func, bcol in ((0, AF.Sigmoid, 0), (2, AF.Tanh, 2), (3, AF.Sigmoid, 3)):
                ps = psB.tile([P, P], f32, tag="psb")
                nc.tensor.matmul(
                    out=ps[:],
                    lhsT=wiht_sb[:, m * P : (m + 1) * P],
                    rhs=wt_sb[:],
                    start=True,
                    stop=True,
                )
                sb = constp.tile([P, P], f32, tag=f"gate{m}")
                nc.scalar.activation(
                    out=sb[:], in_=ps[:], func=func, bias=bias_sb[:, bcol : bcol + 1]
                )
                gate_sb[m] = sb
            cT = constp.tile([P, P], f32, tag="cT")
            nc.vector.tensor_mul(out=cT[:], in0=gate_sb[0][:], in1=gate_sb[2][:])
            tcT = constp.tile([P, P], f32, tag="tcT")
            nc.scalar.activation(out=tcT[:], in_=cT[:], func=AF.Tanh)
            wnT = constp.tile([P, P], f32, tag="wnT")
            nc.vector.tensor_mul(out=wnT[:], in0=gate_sb[3][:], in1=tcT[:])
            wn_ps = psB.tile([P, P], f32, tag="psb")
            nc.tensor.transpose(out=wn_ps[:], in_=wnT[:], identity=ident[:])
            wn_sb = constp.tile([P, P], bf16, tag="wn")
            nc.vector.tensor_copy(out=wn_sb[:], in_=wn_ps[:])

            # --- main: chunks of blocks; self pass + 2 gather passes ---
            do_pe = variant in ("full", "noscat", "deadsoh")
            do_dve = variant in ("full", "gatherdve", "deadsoh")
            use_soh = variant == "full"
            do_gather = variant != "nogather"

            def emit_main(_iv=None):
              b0 = 0  # first block of chunk
              s_tile = 0  # global edge-tile cursor
              s_idx = 0  # global gidx column cursor (int16 cols)
              n_call = 0  # gather call counter (queue round-robin)
              for cs in chunk_sizes:
                if do_pe:
                    aggs = [
                        psA.tile([P, P], f32, name=f"agg{i}", tag="agg")
                        for i in range(cs)
                    ]
                    # self-loop pass: aggT[b] = (dinv*x_block)^T; the second
                    # dinv factor is applied at the flush
                    for i in range(cs):
                        b = b0 + i
                        xs = selfp.tile([P, P], bf16, tag="xself")
                        nc.sync.dma_start(
                            out=xs[:], in_=xself[b * P : (b + 1) * P, :]
                        )
                        nc.tensor.matmul(
                            out=aggs[i][:], lhsT=xs[:], rhs=identb[:],
                            start=True, stop=False,
                        )
                # gather passes
                for r in range(2):
                    sec_tiles = cs * t_r[r]
                    num = sec_tiles * P
                    idx_t = metap.tile([P, sec_tiles * 8], i16, tag="idx")
                    nc.sync.dma_start(
                        out=idx_t[:], in_=gidx[:, s_idx : s_idx + sec_tiles * 8]
                    )
                    stag = stagp.tile([P, sec_tiles * P], bf16, tag="stag")
                    for c0 in range(0, sec_tiles, CALL_T):
                        if not do_gather:
                            break
                        ct = min(CALL_T, sec_tiles - c0)
                        nc.gpsimd.dma_gather(
                            out_ap=stag[:, c0 * P : (c0 + ct) * P].rearrange(
                                "p (t c) -> p t c", t=ct
                            ),
                            in_ap=xsrc[bases[r] :, :],
                            idxs_ap=idx_t[:, c0 * 8 : (c0 + ct) * 8],
                            num_idxs=ct * P,
                            num_idxs_reg=ct * P,
                            elem_size=P,
                            queue_num=n_call % nq,
                        )
                        n_call += 1
                    if not (do_pe or do_dve):
                        s_tile += sec_tiles
                        s_idx += sec_tiles * 8
                        continue
                    if do_dve:
                        # host-precomputed one-hot scatter tiles, streamed via
                        # HWDGE (no Q7 work, no DVE: both would starve the
                        # SWDGE gather descriptor generation)
                        s_oh = workp.tile([P, sec_tiles * P], mybir.dt.float8e4, tag="soh")
                        nc.sync.dma_start(
                            out=s_oh[:],
                            in_=gsoh[:, s_tile * P : (s_tile + sec_tiles) * P],
                        )
                    for i in range(cs):
                        last_of_block = (r == 1)
                        for t in range(t_r[r]):
                            j = i * t_r[r] + t  # tile within section
                            if do_pe:
                                nc.tensor.matmul(
                                    out=aggs[i][:],
                                    lhsT=stag[:, j * P : (j + 1) * P],
                                    rhs=s_oh[:, j * P : (j + 1) * P]
                                    if use_soh
                                    else identb[:],
                                    start=False,
                                    stop=(last_of_block and t == t_r[r] - 1),
                                )
                    s_tile += sec_tiles
                    s_idx += sec_tiles * 8
                # flush chunk
                for i in range(cs if do_pe else 0):
                    b = b0 + i
                    agg_sb = osbp.tile([P, P], bf16, tag="aggsb")
                    nc.scalar.activation(out=agg_sb[:], in_=aggs[i][:], func=AF.Copy)
                    y_ps = psB.tile([P, P], f32, tag="psb")
                    nc.tensor.matmul(
                        out=y_ps[:], lhsT=agg_sb[:], rhs=wn_sb[:],
                        start=True, stop=True,
                    )
                    y_sb = osbp.tile([P, P], f32, tag="ysb")
                    nc.scalar.activation(
                        out=y_sb[:], in_=y_ps[:], func=AF.Copy,
                        scale=d2_sb[:, b : b + 1],
                    )
                    nc.sync.dma_start(
                        out=out[b * P : (b + 1) * P, :], in_=y_sb[:]
                    )
                b0 += cs

            if reps > 1:
                with tc.For_i(0, reps, 1):
                    emit_main()
            else:
                emit_main()

    nc.finalize()
    return nc


def kernel(**inputs) -> np.ndarray:
    from concourse.bass_utils import run_bass_kernel_spmd

    x = inputs["x"]
    n = x.shape[0]
    in_maps, meta = prep_inputs(
        x,
        inputs["edge_index"],
        inputs["weight"],
        inputs["w_ih"],
        inputs["b_ih"],
        inputs["b_hh"],
        n=n,
    )
    nc = build_program(meta)
    res = run_bass_kernel_spmd(nc, in_maps, list(range(N_CORES)))
    full = np.concatenate([r["out"] for r in res.results], axis=0)
    return np.ascontiguousarray(full[:n])



# revision 59
# speedup vs baseline: 1.1178x; 1.0194x over previous
"""EvolveGCN-O forward pass on 8 Trainium2 NeuronCores (Bass/Tile).

Math (reference):
    w_new = LSTM-evolve(weight; w_ih, b_ih+b_hh)          # [C, C]
    out   = D^-1/2 (A + I) D^-1/2  X  w_new               # [N, C]

Device strategy (per sharding hint: edges + scatter targets sharded):
  * Destination nodes padded to NPAD (multiple of 128*8); 128-node
    blocks; each core owns nbc consecutive blocks, processed in chunks
    of 7 (7 PSUM banks accumulate 7 blocks; the 8th bank holds Y).
  * Self-loop term dinv[i]^2 x[i]: contiguous x rows loaded directly,
    scaled on the scalar engine, transposed into the block's PSUM
    accumulator via an identity matmul (start of each accumulation).
  * Edges: host sorts by dst block and splits by source range (the
    dma_gather index is a SIGNED int16 offset from the call's base row,
    so one call reaches a 65536-row window -> 2 ranges cover N=100k).
    Every (block, range) segment is padded to a uniform tile count
    (norm=0 padding, >=1 slack slot so no call ends on a negative
    index, which the ucode would drop).
  * Per edge tile of 128: gpsimd.dma_gather stages rows x[src] (one
    call per (chunk, range) section, ~4-6k rows); scalar engine scales
    by the per-edge norm dinv[src]*dinv[dst]; vector engine builds the
    one-hot dst selector via is_equal(dstl, iota); PE accumulates
    aggT += M^T @ S. Per block: Y = aggT^T @ w_new, DMA out.
  * w_new computed on-device (3 matmuls + activations), redundantly per
    core. No collectives: block ownership makes outputs disjoint.
"""
import sys

for _p in ("/opt/trn_rl_repo", "/root/.axon_site/_ro/trn_rl_repo"):
    if _p not in sys.path:
        sys.path.append(_p)

import ml_dtypes
import numpy as np

BF16 = ml_dtypes.bfloat16

N, C, E = 100000, 128, 1600000  # problem shape (hardcoded per spec)
P = 128
N_CORES = 8
CHUNK = 7  # blocks per PSUM-resident chunk (7 psA banks + psB = 8)
IDX_WIN = 32768  # int16 signed reach below/above base
import os as _os

CALL_T = int(_os.environ.get("CALL_T", "8"))  # edge tiles per dma_gather call
NQ = 4  # SWDGE queues: gather desc-gen runs on Q7 core pair 2q/2q+1


def _cdiv(a, b):
    return -(-a // b)


def prep_inputs(x, edge_index, weight, w_ih, b_ih, b_hh, n=N):
    """Host-side sharding/index prep.

    Returns (in_maps, meta) where meta = (t_r tuple, nbc, chunk_sizes).
    """
    x = np.ascontiguousarray(np.asarray(x, dtype=np.float32))
    ei = np.asarray(edge_index)
    src_e = ei[0].astype(np.int64)
    dst_e = ei[1].astype(np.int64)

    npad = _cdiv(n, P * N_CORES) * P * N_CORES
    nb = npad // P
    nbc = nb // N_CORES

    # degrees include self loops
    deg = (np.bincount(dst_e, minlength=n) + 1).astype(np.float32)
    dinv = (1.0 / np.sqrt(deg)).astype(np.float32)
    d2 = np.zeros(npad, np.float32)  # dinv[dst], applied at the flush
    d2[:n] = dinv

    # dinv[src] folded into the gathered rows host-side
    xpad = np.zeros((npad, C), BF16)
    xpad[:n] = (x * dinv[:, None]).astype(BF16)

    # source ranges
    split = npad // 2
    bases = (max(0, split - IDX_WIN), max(0, npad - IDX_WIN))
    los = (0, split)
    his = (split, npad)
    rng_of = (src_e >= split).astype(np.int64)

    # sort edges by (block, range) then pack
    blk = dst_e >> 7
    order = np.argsort(blk * 2 + rng_of, kind="stable")
    srcs = src_e[order]
    dsts = dst_e[order]
    rngs = rng_of[order]
    blks = blk[order]

    # per-(block, range) counts -> uniform tile counts. +4 slack slots so
    # every cell keeps spare padding for the call-final swap below.
    cell = blks * 2 + rngs
    counts = np.bincount(cell, minlength=nb * 2).reshape(nb, 2)
    t_r = tuple(int(_cdiv(int(counts[:, r].max()) + 4, P)) for r in range(2))
    cap = (t_r[0] * P, t_r[1] * P)

    # chunk structure (uniform across cores)
    chunk_sizes = [min(CHUNK, nbc - i) for i in range(0, nbc, CHUNK)]

    # pack edges into per-(block, range) padded slots
    cell_cap = np.array([cap[0], cap[1]], np.int64)
    cell_starts = np.zeros(nb * 2 + 1, np.int64)
    np.cumsum(counts.reshape(-1), out=cell_starts[1:])
    pos_in_cell = np.arange(len(srcs)) - cell_starts[cell]
    slot = cell * 0  # placeholder
    # flat slot index: block-major [b][r][slot]
    cell_base = np.zeros(nb * 2, np.int64)
    cell_base[0::2] = np.arange(nb) * (cap[0] + cap[1])
    cell_base[1::2] = cell_base[0::2] + cap[0]
    flat = cell_base[cell] + pos_in_cell

    tot = nb * (cap[0] + cap[1])
    idx_all = np.zeros(tot, np.int32)  # padding idx = 0 (valid row at base)
    # padding dstl = 200: matches no iota lane -> one-hot column all-zero
    dstl_all = np.full(tot, 200.0, np.float32)
    idx_all[flat] = (srcs - np.array(bases)[rngs]).astype(np.int32)
    dstl_all[flat] = (dsts & (P - 1)).astype(np.float32)


    wt = np.ascontiguousarray(np.asarray(weight, np.float32).T)
    wiht = np.ascontiguousarray(np.asarray(w_ih, np.float32).T)
    bsum = (
        (np.asarray(b_ih, np.float32) + np.asarray(b_hh, np.float32))
        .reshape(4, C)
        .T.copy()
    )

    per_blk = cap[0] + cap[1]
    in_maps = []
    for m in range(N_CORES):
        lo_b = m * nbc
        seg = slice(lo_b * per_blk, (lo_b + nbc) * per_blk)
        idx_c = idx_all[seg].reshape(nbc, per_blk)
        dstl_c = dstl_all[seg].reshape(nbc, per_blk)

        # build per-(chunk, range) sections: [sections] each a flat idx list
        gidx_secs = []
        gdstl_secs = []
        b0 = 0
        for cs in chunk_sizes:
            for r in range(2):
                off = 0 if r == 0 else cap[0]
                sec_idx = idx_c[b0 : b0 + cs, off : off + cap[r]].reshape(-1).copy()
                sec_dstl = (
                    dstl_c[b0 : b0 + cs, off : off + cap[r]].reshape(-1).copy()
                )
                # the gather ucode DROPS a trailing negative index, so the
                # final slot of every call must be >= 0: swap offending real
                # edges with a padding slot of the SAME (block,range) cell.
                sec_tiles = cs * t_r[r]
                call_ts = [CALL_T] * (sec_tiles // CALL_T)
                if sec_tiles % CALL_T:
                    call_ts.append(sec_tiles % CALL_T)
                ends = np.cumsum(np.array(call_ts)) * P - 1  # call-final slots
                end_set = set(int(e) for e in ends)
                for s in ends:
                    s = int(s)
                    if sec_idx[s] >= 0:
                        continue
                    k = s // cap[r]  # cell (block) within section
                    cnt = int(counts[lo_b + b0 + k, r])
                    for p in range(k * cap[r] + cnt, (k + 1) * cap[r]):
                        if p not in end_set:
                            for arr in (sec_idx, sec_dstl):
                                arr[s], arr[p] = arr[p], arr[s]
                            break
                    else:
                        raise RuntimeError("no swap slot for call-final pad")
                gidx_secs.append(sec_idx)
                gdstl_secs.append(sec_dstl)
            b0 += cs

        # idx wrap per CALL: i -> [i%16, i//16], replicated x8 across
        # partitions; concatenate calls/sections along free dim
        def wrap16(a):
            w = np.ascontiguousarray(a.reshape(-1, 16).T.astype(np.int16))
            return np.tile(w, (8, 1))  # [128, len/16]

        gidx_cols = []
        for sec in gidx_secs:
            st = len(sec) // P
            for c0 in range(0, st, CALL_T):
                ct = min(CALL_T, st - c0)
                gidx_cols.append(wrap16(sec[c0 * P : (c0 + ct) * P]))
        gidx = np.concatenate(gidx_cols, axis=1)

        # per-tile metadata, partition-major per section: [128, tiles]
        def tilemeta(secs):
            cols = [s.reshape(-1, P).T for s in secs]  # [128, tiles_sec]
            return np.ascontiguousarray(np.concatenate(cols, axis=1))

        # one-hot scatter tiles precomputed host-side, streamed via HWDGE:
        # oh[p, j*128+l] = 1 iff tile j's slot p targets lane l (pad -> all 0)
        gdstl = tilemeta(gdstl_secs)  # [128, n_tiles] fp32
        gsoh = (
            (gdstl[:, :, None] == np.arange(P, dtype=np.float32)[None, None, :])
            .astype(ml_dtypes.float8_e4m3)
            .reshape(P, -1)
        )

        d2_c = d2[lo_b * P : (lo_b + nbc) * P].reshape(nbc, P).T.copy()  # [128, nbc]

        in_maps.append(
            dict(
                xsrc=xpad,
                xself=np.ascontiguousarray(xpad[lo_b * P : (lo_b + nbc) * P]),
                gidx=np.ascontiguousarray(gidx),
                gsoh=np.ascontiguousarray(gsoh),
                d2=d2_c,
                wt=wt,
                wiht=wiht,
                bias=bsum,
            )
        )
    meta = (t_r, nbc, chunk_sizes, bases, npad)
    return in_maps, meta


def build_program(meta, reps=1, variant="full", nq=NQ):
    import contextlib

    import concourse.bacc as bacc
    import concourse.mybir as mybir
    import concourse.tile as tile
    from concourse.masks import make_identity

    t_r, nbc, chunk_sizes, bases, npad = meta
    f32 = mybir.dt.float32
    bf16 = mybir.dt.bfloat16
    i16 = mybir.dt.int16
    AF = mybir.ActivationFunctionType
    ALU = mybir.AluOpType

    n_tiles = nbc * (t_r[0] + t_r[1])  # edge tiles per core
    idx_w = n_tiles * P // 16  # gidx free dim
    nsec = 2 * len(chunk_sizes)

    nc = bacc.Bacc("TRN2", num_swdge_queues=nq)
    xsrc = nc.declare_dram_parameter("xsrc", [npad, C], bf16, isOutput=False)
    xself = nc.declare_dram_parameter("xself", [nbc * P, C], bf16, isOutput=False)
    gidx = nc.declare_dram_parameter("gidx", [P, idx_w], i16, isOutput=False)
    gsoh = nc.declare_dram_parameter("gsoh", [P, n_tiles * P], mybir.dt.float8e4, isOutput=False)
    d2 = nc.declare_dram_parameter("d2", [P, nbc], f32, isOutput=False)
    wt = nc.declare_dram_parameter("wt", [P, P], f32, isOutput=False)
    wiht = nc.declare_dram_parameter("wiht", [P, 4 * C], f32, isOutput=False)
    bias = nc.declare_dram_parameter("bias", [P, 4], f32, isOutput=False)
    out = nc.declare_dram_parameter("out", [nbc * P, C], f32, isOutput=True)

    with tile.TileContext(nc) as tc:
        with (
            tc.tile_pool(name="const", bufs=1) as constp,
            tc.tile_pool(name="stag", bufs=4) as stagp,
            tc.tile_pool(name="meta", bufs=2) as metap,
            tc.tile_pool(name="work", bufs=3) as workp,
            tc.tile_pool(name="selfx", bufs=8) as selfp,
            tc.tile_pool(name="psA", bufs=CHUNK, space="PSUM") as psA,
            tc.tile_pool(name="psB", bufs=1, space="PSUM") as psB,
            tc.tile_pool(name="osb", bufs=3) as osbp,
        ):

            wt_sb = constp.tile([P, P], f32, tag="wt")
            nc.sync.dma_start(out=wt_sb[:], in_=wt[:])
            wiht_sb = constp.tile([P, 4 * C], f32, tag="wiht")
            nc.sync.dma_start(out=wiht_sb[:], in_=wiht[:])
            bias_sb = constp.tile([P, 4], f32, tag="bias")
            nc.sync.dma_start(out=bias_sb[:], in_=bias[:])
            d2_sb = constp.tile([P, nbc], f32, tag="d2")
            nc.sync.dma_start(out=d2_sb[:], in_=d2[:])
            ident = constp.tile([P, P], f32, tag="ident")
            make_identity(nc, ident[:])
            identb = constp.tile([P, P], bf16, tag="identb")
            make_identity(nc, identb[:])

            # --- LSTM single step -> evolved weight w_new ---
            gate_sb = {}
            for m, func, bcol in ((0, AF.Sigmoid, 0), (2, AF.Tanh, 2), (3, AF.Sigmoid, 3)):
                ps = psB.tile([P, P], f32, tag="psb")
                nc.tensor.matmul(
                    out=ps[:],
                    lhsT=wiht_sb[:, m * P : (m + 1) * P],
                    rhs=wt_sb[:],
                    start=True,
                    stop=True,
                )
                sb = constp.tile([P, P], f32, tag=f"gate{m}")
                nc.scalar.activation(
                    out=sb[:], in_=ps[:], func=func, bias=bias_sb[:, bcol : bcol + 1]
                )
                gate_sb[m] = sb
            cT = constp.tile([P, P], f32, tag="cT")
            nc.vector.tensor_mul(out=cT[:], in0=gate_sb[0][:], in1=gate_sb[2][:])
            tcT = constp.tile([P, P], f32, tag="tcT")
            nc.scalar.activation(out=tcT[:], in_=cT[:], func=AF.Tanh)
            wnT = constp.tile([P, P], f32, tag="wnT")
            nc.vector.tensor_mul(out=wnT[:], in0=gate_sb[3][:], in1=tcT[:])
            wn_ps = psB.tile([P, P], f32, tag="psb")
            nc.tensor.transpose(out=wn_ps[:], in_=wnT[:], identity=ident[:])
            wn_sb = constp.tile([P, P], bf16, tag="wn")
            nc.vector.tensor_copy(out=wn_sb[:], in_=wn_ps[:])

            # --- main: chunks of blocks; self pass + 2 gather passes ---
            do_pe = variant in ("full", "noscat", "deadsoh")
            do_dve = variant in ("full", "gatherdve", "deadsoh")
            use_soh = variant == "full"
            do_gather = variant != "nogather"

            def emit_main(_iv=None):
              b0 = 0  # first block of chunk
              s_tile = 0  # global edge-tile cursor
              s_idx = 0  # global gidx column cursor (int16 cols)
              n_call = 0  # gather call counter (queue round-robin)
              for cs in chunk_sizes:
                if do_pe:
                    aggs = [
                        psA.tile([P, P], f32, name=f"agg{i}", tag="agg")
                        for i in range(cs)
                    ]
                    # self-loop pass: aggT[b] = (dinv*x_block)^T; the second
                    # dinv factor is applied at the flush
                    for i in range(cs):
                        b = b0 + i
                        xs = selfp.tile([P, P], bf16, tag="xself")
                        nc.sync.dma_start(
                            out=xs[:], in_=xself[b * P : (b + 1) * P, :]
                        )
                        nc.tensor.matmul(
                            out=aggs[i][:], lhsT=xs[:], rhs=identb[:],
                            start=True, stop=False,
                        )
                # gather passes
                for r in range(2):
                    sec_tiles = cs * t_r[r]
                    num = sec_tiles * P
                    idx_t = metap.tile([P, sec_tiles * 8], i16, tag="idx")
                    nc.sync.dma_start(
                        out=idx_t[:], in_=gidx[:, s_idx : s_idx + sec_tiles * 8]
                    )
                    stag = stagp.tile([P, sec_tiles * P], bf16, tag="stag")
                    for c0 in range(0, sec_tiles, CALL_T):
                        if not do_gather:
                            break
                        ct = min(CALL_T, sec_tiles - c0)
                        nc.gpsimd.dma_gather(
                            out_ap=stag[:, c0 * P : (c0 + ct) * P].rearrange(
                                "p (t c) -> p t c", t=ct
                            ),
                            in_ap=xsrc[bases[r] :, :],
                            idxs_ap=idx_t[:, c0 * 8 : (c0 + ct) * 8],
                            num_idxs=ct * P,
                            num_idxs_reg=ct * P,
                            elem_size=P,
                            queue_num=n_call % nq,
                        )
                        n_call += 1
                    if not (do_pe or do_dve):
                        s_tile += sec_tiles
                        s_idx += sec_tiles * 8
                        continue
                    if do_dve:
                        # host-precomputed one-hot scatter tiles, streamed via
                        # HWDGE (no Q7 work, no DVE: both would starve the
                        # SWDGE gather descriptor generation)
                        s_oh = workp.tile([P, sec_tiles * P], mybir.dt.float8e4, tag="soh")
                        nc.sync.dma_start(
                            out=s_oh[:],
                            in_=gsoh[:, s_tile * P : (s_tile + sec_tiles) * P],
                        )
                    for i in range(cs):
                        last_of_block = (r == 1)
                        for t in range(t_r[r]):
                            j = i * t_r[r] + t  # tile within section
                            if do_pe:
                                nc.tensor.matmul(
                                    out=aggs[i][:],
                                    lhsT=stag[:, j * P : (j + 1) * P],
                                    rhs=s_oh[:, j * P : (j + 1) * P]
                                    if use_soh
                                    else identb[:],
                                    start=False,
                                    stop=(last_of_block and t == t_r[r] - 1),
                                )
                    s_tile += sec_tiles
                    s_idx += sec_tiles * 8
                # flush chunk
                for i in range(cs if do_pe else 0):
                    b = b0 + i
                    agg_sb = osbp.tile([P, P], bf16, tag="aggsb")
                    nc.scalar.activation(out=agg_sb[:], in_=aggs[i][:], func=AF.Copy)
                    y_ps = psB.tile([P, P], f32, tag="psb")
                    nc.tensor.matmul(
                        out=y_ps[:], lhsT=agg_sb[:], rhs=wn_sb[:],
                        start=True, stop=True,
                    )
                    y_sb = osbp.tile([P, P], f32, tag="ysb")
                    nc.scalar.activation(
                        out=y_sb[:], in_=y_ps[:], func=AF.Copy,
                        scale=d2_sb[:, b : b + 1],
                    )
                    nc.sync.dma_start(
                        out=out[b * P : (b + 1) * P, :], in_=y_sb[:]
                    )
                b0 += cs

            if reps > 1:
                with tc.For_i(0, reps, 1):
                    emit_main()
            else:
                emit_main()

    nc.finalize()
    return nc


def kernel(**inputs) -> np.ndarray:
    from concourse.bass_utils import run_bass_kernel_spmd

    x = inputs["x"]
    n = x.shape[0]
    in_maps, meta = prep_inputs(
        x,
        inputs["edge_index"],
        inputs["weight"],
        inputs["w_ih"],
        inputs["b_ih"],
        inputs["b_hh"],
        n=n,
    )
    nc = build_program(meta)
    res = run_bass_kernel_spmd(nc, in_maps, list(range(N_CORES)))
    full = np.concatenate([r["out"] for r in res.results], axis=0)
    return np.ascontiguousarray(full[:n])



# revision 60
# speedup vs baseline: 1.3422x; 1.2007x over previous
"""EvolveGCN-O forward pass on 8 Trainium2 NeuronCores (Bass/Tile).

Math (reference):
    w_new = LSTM-evolve(weight; w_ih, b_ih+b_hh)          # [C, C]
    out   = D^-1/2 (A + I) D^-1/2  X  w_new               # [N, C]

Device strategy (per sharding hint: edges + scatter targets sharded):
  * Destination nodes padded to NPAD (multiple of 128*8); 128-node
    blocks; each core owns nbc consecutive blocks, processed in chunks
    of 7 (7 PSUM banks accumulate 7 blocks; the 8th bank holds Y).
  * Self-loop term dinv[i]^2 x[i]: contiguous x rows loaded directly,
    scaled on the scalar engine, transposed into the block's PSUM
    accumulator via an identity matmul (start of each accumulation).
  * Edges: host sorts by dst block and splits by source range (the
    dma_gather index is a SIGNED int16 offset from the call's base row,
    so one call reaches a 65536-row window -> 2 ranges cover N=100k).
    Every (block, range) segment is padded to a uniform tile count
    (norm=0 padding, >=1 slack slot so no call ends on a negative
    index, which the ucode would drop).
  * Per edge tile of 128: gpsimd.dma_gather stages rows x[src] (one
    call per (chunk, range) section, ~4-6k rows); scalar engine scales
    by the per-edge norm dinv[src]*dinv[dst]; vector engine builds the
    one-hot dst selector via is_equal(dstl, iota); PE accumulates
    aggT += M^T @ S. Per block: Y = aggT^T @ w_new, DMA out.
  * w_new computed on-device (3 matmuls + activations), redundantly per
    core. No collectives: block ownership makes outputs disjoint.
"""
import sys

for _p in ("/opt/trn_rl_repo", "/root/.axon_site/_ro/trn_rl_repo"):
    if _p not in sys.path:
        sys.path.append(_p)

import ml_dtypes
import numpy as np

BF16 = ml_dtypes.bfloat16

N, C, E = 100000, 128, 1600000  # problem shape (hardcoded per spec)
P = 128
N_CORES = 8
CHUNK = 7  # blocks per PSUM-resident chunk (7 psA banks + psB = 8)
IDX_WIN = 32768  # int16 signed reach below/above base
import os as _os

CALL_T = int(_os.environ.get("CALL_T", "8"))  # edge tiles per dma_gather call
NQ = 4  # SWDGE queues: gather desc-gen runs on Q7 core pair 2q/2q+1


def _cdiv(a, b):
    return -(-a // b)


def prep_inputs(x, edge_index, weight, w_ih, b_ih, b_hh, n=N):
    """Host-side sharding/index prep.

    Returns (in_maps, meta) where meta = (t_r tuple, nbc, chunk_sizes).
    """
    x = np.ascontiguousarray(np.asarray(x, dtype=np.float32))
    ei = np.asarray(edge_index)
    src_e = ei[0].astype(np.int64)
    dst_e = ei[1].astype(np.int64)

    npad = _cdiv(n, P * N_CORES) * P * N_CORES
    nb = npad // P
    nbc = nb // N_CORES

    # degrees include self loops
    deg = (np.bincount(dst_e, minlength=n) + 1).astype(np.float32)
    dinv = (1.0 / np.sqrt(deg)).astype(np.float32)
    d2 = np.zeros(npad, np.float32)  # dinv[dst], applied at the flush
    d2[:n] = dinv

    # dinv[src] folded into the gathered rows host-side
    xpad = np.zeros((npad, C), BF16)
    xpad[:n] = (x * dinv[:, None]).astype(BF16)

    # source ranges
    split = npad // 2
    bases = (max(0, split - IDX_WIN), max(0, npad - IDX_WIN))
    los = (0, split)
    his = (split, npad)
    rng_of = (src_e >= split).astype(np.int64)

    # sort edges by (block, range) then pack
    blk = dst_e >> 7
    order = np.argsort(blk * 2 + rng_of, kind="stable")
    srcs = src_e[order]
    dsts = dst_e[order]
    rngs = rng_of[order]
    blks = blk[order]

    # per-(block, range) counts -> uniform tile counts. +4 slack slots so
    # every cell keeps spare padding for the call-final swap below.
    cell = blks * 2 + rngs
    counts = np.bincount(cell, minlength=nb * 2).reshape(nb, 2)
    t_r = tuple(int(_cdiv(int(counts[:, r].max()) + 4, P)) for r in range(2))
    cap = (t_r[0] * P, t_r[1] * P)

    # chunk structure (uniform across cores)
    chunk_sizes = [min(CHUNK, nbc - i) for i in range(0, nbc, CHUNK)]

    # pack edges into per-(block, range) padded slots
    cell_cap = np.array([cap[0], cap[1]], np.int64)
    cell_starts = np.zeros(nb * 2 + 1, np.int64)
    np.cumsum(counts.reshape(-1), out=cell_starts[1:])
    pos_in_cell = np.arange(len(srcs)) - cell_starts[cell]
    slot = cell * 0  # placeholder
    # flat slot index: block-major [b][r][slot]
    cell_base = np.zeros(nb * 2, np.int64)
    cell_base[0::2] = np.arange(nb) * (cap[0] + cap[1])
    cell_base[1::2] = cell_base[0::2] + cap[0]
    flat = cell_base[cell] + pos_in_cell

    tot = nb * (cap[0] + cap[1])
    idx_all = np.zeros(tot, np.int32)  # padding idx = 0 (valid row at base)
    # padding dstl = 200: matches no iota lane -> one-hot column all-zero
    dstl_all = np.full(tot, 200.0, np.float32)
    idx_all[flat] = (srcs - np.array(bases)[rngs]).astype(np.int32)
    dstl_all[flat] = (dsts & (P - 1)).astype(np.float32)


    wt = np.ascontiguousarray(np.asarray(weight, np.float32).T)
    wiht = np.ascontiguousarray(np.asarray(w_ih, np.float32).T)
    bsum = (
        (np.asarray(b_ih, np.float32) + np.asarray(b_hh, np.float32))
        .reshape(4, C)
        .T.copy()
    )

    per_blk = cap[0] + cap[1]
    in_maps = []
    for m in range(N_CORES):
        lo_b = m * nbc
        seg = slice(lo_b * per_blk, (lo_b + nbc) * per_blk)
        idx_c = idx_all[seg].reshape(nbc, per_blk)
        dstl_c = dstl_all[seg].reshape(nbc, per_blk)

        # build per-(chunk, range) sections: [sections] each a flat idx list
        gidx_secs = []
        gdstl_secs = []
        b0 = 0
        for cs in chunk_sizes:
            for r in range(2):
                off = 0 if r == 0 else cap[0]
                sec_idx = idx_c[b0 : b0 + cs, off : off + cap[r]].reshape(-1).copy()
                sec_dstl = (
                    dstl_c[b0 : b0 + cs, off : off + cap[r]].reshape(-1).copy()
                )
                # the gather ucode DROPS a trailing negative index, so the
                # final slot of every call must be >= 0: swap offending real
                # edges with a padding slot of the SAME (block,range) cell.
                sec_tiles = cs * t_r[r]
                call_ts = [CALL_T] * (sec_tiles // CALL_T)
                if sec_tiles % CALL_T:
                    call_ts.append(sec_tiles % CALL_T)
                ends = np.cumsum(np.array(call_ts)) * P - 1  # call-final slots
                end_set = set(int(e) for e in ends)
                for s in ends:
                    s = int(s)
                    if sec_idx[s] >= 0:
                        continue
                    k = s // cap[r]  # cell (block) within section
                    cnt = int(counts[lo_b + b0 + k, r])
                    for p in range(k * cap[r] + cnt, (k + 1) * cap[r]):
                        if p not in end_set:
                            for arr in (sec_idx, sec_dstl):
                                arr[s], arr[p] = arr[p], arr[s]
                            break
                    else:
                        raise RuntimeError("no swap slot for call-final pad")
                gidx_secs.append(sec_idx)
                gdstl_secs.append(sec_dstl)
            b0 += cs

        # idx wrap per CALL: i -> [i%16, i//16], replicated x8 across
        # partitions; concatenate calls/sections along free dim
        def wrap16(a):
            w = np.ascontiguousarray(a.reshape(-1, 16).T.astype(np.int16))
            return np.tile(w, (8, 1))  # [128, len/16]

        gidx_cols = []
        for sec in gidx_secs:
            st = len(sec) // P
            for c0 in range(0, st, CALL_T):
                ct = min(CALL_T, st - c0)
                gidx_cols.append(wrap16(sec[c0 * P : (c0 + ct) * P]))
        gidx = np.concatenate(gidx_cols, axis=1)

        # per-tile metadata, partition-major per section: [128, tiles]
        def tilemeta(secs):
            cols = [s.reshape(-1, P).T for s in secs]  # [128, tiles_sec]
            return np.ascontiguousarray(np.concatenate(cols, axis=1))

        # one-hot scatter tiles precomputed host-side, streamed via HWDGE:
        # oh[p, j*128+l] = 1 iff tile j's slot p targets lane l (pad -> all 0)
        gdstl = tilemeta(gdstl_secs)  # [128, n_tiles] fp32
        gsoh = (
            (gdstl[:, :, None] == np.arange(P, dtype=np.float32)[None, None, :])
            .astype(ml_dtypes.float8_e4m3)
            .reshape(P, -1)
        )

        d2_c = d2[lo_b * P : (lo_b + nbc) * P].reshape(nbc, P).T.copy()  # [128, nbc]

        in_maps.append(
            dict(
                xsrc=xpad,
                xself=np.ascontiguousarray(xpad[lo_b * P : (lo_b + nbc) * P]),
                gidx=np.ascontiguousarray(gidx),
                gsoh=np.ascontiguousarray(gsoh),
                d2=d2_c,
                wt=wt,
                wiht=wiht,
                bias=bsum,
            )
        )
    meta = (t_r, nbc, chunk_sizes, bases, npad)
    return in_maps, meta


def build_program(meta, reps=1, variant="full", nq=NQ):
    import contextlib

    import concourse.bacc as bacc
    import concourse.mybir as mybir
    import concourse.tile as tile
    from concourse.masks import make_identity

    t_r, nbc, chunk_sizes, bases, npad = meta
    f32 = mybir.dt.float32
    bf16 = mybir.dt.bfloat16
    i16 = mybir.dt.int16
    AF = mybir.ActivationFunctionType
    ALU = mybir.AluOpType

    n_tiles = nbc * (t_r[0] + t_r[1])  # edge tiles per core
    idx_w = n_tiles * P // 16  # gidx free dim
    nsec = 2 * len(chunk_sizes)

    nc = bacc.Bacc("TRN2", num_swdge_queues=nq)
    xsrc = nc.declare_dram_parameter("xsrc", [npad, C], bf16, isOutput=False)
    xself = nc.declare_dram_parameter("xself", [nbc * P, C], bf16, isOutput=False)
    gidx = nc.declare_dram_parameter("gidx", [P, idx_w], i16, isOutput=False)
    gsoh = nc.declare_dram_parameter("gsoh", [P, n_tiles * P], mybir.dt.float8e4, isOutput=False)
    d2 = nc.declare_dram_parameter("d2", [P, nbc], f32, isOutput=False)
    wt = nc.declare_dram_parameter("wt", [P, P], f32, isOutput=False)
    wiht = nc.declare_dram_parameter("wiht", [P, 4 * C], f32, isOutput=False)
    bias = nc.declare_dram_parameter("bias", [P, 4], f32, isOutput=False)
    out = nc.declare_dram_parameter("out", [nbc * P, C], f32, isOutput=True)

    with tile.TileContext(nc) as tc:
        with (
            tc.tile_pool(name="const", bufs=1) as constp,
            tc.tile_pool(name="stag", bufs=4) as stagp,
            tc.tile_pool(name="meta", bufs=4) as metap,
            tc.tile_pool(name="work", bufs=3) as workp,
            tc.tile_pool(name="selfx", bufs=8) as selfp,
            tc.tile_pool(name="psA", bufs=CHUNK, space="PSUM") as psA,
            tc.tile_pool(name="psB", bufs=1, space="PSUM") as psB,
            tc.tile_pool(name="osb", bufs=3) as osbp,
        ):

            wt_sb = constp.tile([P, P], f32, tag="wt")
            nc.sync.dma_start(out=wt_sb[:], in_=wt[:])
            wiht_sb = constp.tile([P, 4 * C], f32, tag="wiht")
            nc.sync.dma_start(out=wiht_sb[:], in_=wiht[:])
            bias_sb = constp.tile([P, 4], f32, tag="bias")
            nc.sync.dma_start(out=bias_sb[:], in_=bias[:])
            d2_sb = constp.tile([P, nbc], f32, tag="d2")
            nc.sync.dma_start(out=d2_sb[:], in_=d2[:])
            ident = constp.tile([P, P], f32, tag="ident")
            make_identity(nc, ident[:])
            identb = constp.tile([P, P], bf16, tag="identb")
            make_identity(nc, identb[:])

            # --- LSTM single step -> evolved weight w_new ---
            gate_sb = {}
            for m, func, bcol in ((0, AF.Sigmoid, 0), (2, AF.Tanh, 2), (3, AF.Sigmoid, 3)):
                ps = psB.tile([P, P], f32, tag="psb")
                nc.tensor.matmul(
                    out=ps[:],
                    lhsT=wiht_sb[:, m * P : (m + 1) * P],
                    rhs=wt_sb[:],
                    start=True,
                    stop=True,
                )
                sb = constp.tile([P, P], f32, tag=f"gate{m}")
                nc.scalar.activation(
                    out=sb[:], in_=ps[:], func=func, bias=bias_sb[:, bcol : bcol + 1]
                )
                gate_sb[m] = sb
            cT = constp.tile([P, P], f32, tag="cT")
            nc.vector.tensor_mul(out=cT[:], in0=gate_sb[0][:], in1=gate_sb[2][:])
            tcT = constp.tile([P, P], f32, tag="tcT")
            nc.scalar.activation(out=tcT[:], in_=cT[:], func=AF.Tanh)
            wnT = constp.tile([P, P], f32, tag="wnT")
            nc.vector.tensor_mul(out=wnT[:], in0=gate_sb[3][:], in1=tcT[:])
            wn_ps = psB.tile([P, P], f32, tag="psb")
            nc.tensor.transpose(out=wn_ps[:], in_=wnT[:], identity=ident[:])
            wn_sb = constp.tile([P, P], bf16, tag="wn")
            nc.vector.tensor_copy(out=wn_sb[:], in_=wn_ps[:])

            # --- main: chunks of blocks; self pass + 2 gather passes ---
            do_pe = variant in ("full", "noscat", "deadsoh")
            do_dve = variant in ("full", "gatherdve", "deadsoh")
            use_soh = variant == "full"
            do_gather = variant != "nogather"

            def emit_main(_iv=None):
              b0 = 0  # first block of chunk
              s_tile = 0  # global edge-tile cursor
              s_idx = 0  # global gidx column cursor (int16 cols)
              n_call = 0  # gather call counter (queue round-robin)
              for cs in chunk_sizes:
                if do_pe:
                    aggs = [
                        psA.tile([P, P], f32, name=f"agg{i}", tag="agg")
                        for i in range(cs)
                    ]
                    # self-loop pass: aggT[b] = (dinv*x_block)^T; the second
                    # dinv factor is applied at the flush
                    for i in range(cs):
                        b = b0 + i
                        xs = selfp.tile([P, P], bf16, tag="xself")
                        nc.sync.dma_start(
                            out=xs[:], in_=xself[b * P : (b + 1) * P, :]
                        )
                        nc.tensor.matmul(
                            out=aggs[i][:], lhsT=xs[:], rhs=identb[:],
                            start=True, stop=False,
                        )
                # gather passes
                for r in range(2):
                    sec_tiles = cs * t_r[r]
                    num = sec_tiles * P
                    idx_t = metap.tile([P, sec_tiles * 8], i16, tag="idx")
                    nc.sync.dma_start(
                        out=idx_t[:], in_=gidx[:, s_idx : s_idx + sec_tiles * 8]
                    )
                    stag = stagp.tile([P, sec_tiles * P], bf16, tag="stag")
                    for c0 in range(0, sec_tiles, CALL_T):
                        if not do_gather:
                            break
                        ct = min(CALL_T, sec_tiles - c0)
                        nc.gpsimd.dma_gather(
                            out_ap=stag[:, c0 * P : (c0 + ct) * P].rearrange(
                                "p (t c) -> p t c", t=ct
                            ),
                            in_ap=xsrc[bases[r] :, :],
                            idxs_ap=idx_t[:, c0 * 8 : (c0 + ct) * 8],
                            num_idxs=ct * P,
                            num_idxs_reg=ct * P,
                            elem_size=P,
                            queue_num=n_call % nq,
                        )
                        n_call += 1
                    if not (do_pe or do_dve):
                        s_tile += sec_tiles
                        s_idx += sec_tiles * 8
                        continue
                    if do_dve:
                        # host-precomputed one-hot scatter tiles, streamed via
                        # HWDGE (no Q7 work, no DVE: both would starve the
                        # SWDGE gather descriptor generation)
                        s_oh = workp.tile([P, sec_tiles * P], mybir.dt.float8e4, tag="soh")
                        nc.scalar.dma_start(
                            out=s_oh[:],
                            in_=gsoh[:, s_tile * P : (s_tile + sec_tiles) * P],
                        )
                    for i in range(cs):
                        last_of_block = (r == 1)
                        for t in range(t_r[r]):
                            j = i * t_r[r] + t  # tile within section
                            if do_pe:
                                nc.tensor.matmul(
                                    out=aggs[i][:],
                                    lhsT=stag[:, j * P : (j + 1) * P],
                                    rhs=s_oh[:, j * P : (j + 1) * P]
                                    if use_soh
                                    else identb[:],
                                    start=False,
                                    stop=(last_of_block and t == t_r[r] - 1),
                                )
                    s_tile += sec_tiles
                    s_idx += sec_tiles * 8
                # flush chunk
                for i in range(cs if do_pe else 0):
                    b = b0 + i
                    agg_sb = osbp.tile([P, P], bf16, tag="aggsb")
                    nc.scalar.activation(out=agg_sb[:], in_=aggs[i][:], func=AF.Copy)
                    y_ps = psB.tile([P, P], f32, tag="psb")
                    nc.tensor.matmul(
                        out=y_ps[:], lhsT=agg_sb[:], rhs=wn_sb[:],
                        start=True, stop=True,
                    )
                    y_sb = osbp.tile([P, P], f32, tag="ysb")
                    nc.scalar.activation(
                        out=y_sb[:], in_=y_ps[:], func=AF.Copy,
                        scale=d2_sb[:, b : b + 1],
                    )
                    nc.sync.dma_start(
                        out=out[b * P : (b + 1) * P, :], in_=y_sb[:]
                    )
                b0 += cs

            if reps > 1:
                with tc.For_i(0, reps, 1):
                    emit_main()
            else:
                emit_main()

    nc.finalize()
    return nc


def kernel(**inputs) -> np.ndarray:
    from concourse.bass_utils import run_bass_kernel_spmd

    x = inputs["x"]
    n = x.shape[0]
    in_maps, meta = prep_inputs(
        x,
        inputs["edge_index"],
        inputs["weight"],
        inputs["w_ih"],
        inputs["b_ih"],
        inputs["b_hh"],
        n=n,
    )
    nc = build_program(meta)
    res = run_bass_kernel_spmd(nc, in_maps, list(range(N_CORES)))
    full = np.concatenate([r["out"] for r in res.results], axis=0)
    return np.ascontiguousarray(full[:n])



# revision 61
# speedup vs baseline: 1.4144x; 1.0538x over previous
"""EvolveGCN-O forward pass on 8 Trainium2 NeuronCores (Bass/Tile).

Math (reference):
    w_new = LSTM-evolve(weight; w_ih, b_ih+b_hh)          # [C, C]
    out   = D^-1/2 (A + I) D^-1/2  X  w_new               # [N, C]

Device strategy (edges + scatter targets sharded per the hint):
  * Destination nodes padded to NPAD; each core owns nbc consecutive
    128-node blocks, processed in chunks of 7 (7 PSUM banks accumulate
    7 blocks; the 8th bank is scratch).
  * dinv[src] is folded into the gathered rows host-side (xsrc holds
    dinv[i]*x[i] in bf16); dinv[dst] is applied by the per-block flush
    (scalar-engine PSUM->SBUF copy with per-partition scale). No
    per-edge scaling on device.
  * Edges sorted by (dst block, src range); the dma_gather index is a
    SIGNED int16 row offset, so 2 source ranges cover N=100k. Per
    (chunk, range) section, cells (one per block) are packed
    contiguously with span = max-over-cores count + 2 (uniform across
    cores - the program is SPMD). Tiles straddling two cells scatter
    into both blocks via two matmuls.
  * gpsimd.dma_gather stages 256B bf16 rows; desc-gen runs on the Q7
    pair 2q/2q+1, so calls round-robin over 4 SWDGE queues to
    parallelize descriptor generation (the real bottleneck).
  * The dst one-hot scatter tiles are PRECOMPUTED on host (fp8) and
    streamed via the ACT HWDGE ring: building them on device (DVE
    is_equal) stalls SWDGE desc-gen via the shared SBUF port pair, and
    streaming them on the sync ring head-of-line-blocks the idx loads.
  * PE accumulates aggT[ch,lane] += stag_tile^T @ onehot in PSUM fp32;
    per block: Y = aggT^T @ w_new, scaled by dinv[dst], DMA out.
  * w_new computed on-device (3 matmuls + activations), redundantly per
    core. No collectives: block ownership makes outputs disjoint.
"""
import os as _os
import sys

for _p in ("/opt/trn_rl_repo", "/root/.axon_site/_ro/trn_rl_repo"):
    if _p not in sys.path:
        sys.path.append(_p)

import ml_dtypes
import numpy as np

BF16 = ml_dtypes.bfloat16
FP8 = ml_dtypes.float8_e4m3

N, C, E = 100000, 128, 1600000  # problem shape (hardcoded per spec)
P = 128
N_CORES = 8
CHUNK = 7  # blocks per PSUM-resident chunk (7 psA banks + psB = 8)
IDX_WIN = 32768  # int16 signed reach below/above base
CALL_T = int(_os.environ.get("CALL_T", "8"))  # edge tiles per dma_gather call
NQ = 4  # SWDGE queues: gather desc-gen runs on Q7 core pair 2q/2q+1


def _cdiv(a, b):
    return -(-a // b)


def prep_inputs(x, edge_index, weight, w_ih, b_ih, b_hh, n=N):
    """Host-side sharding/index prep. Returns (in_maps, meta)."""
    x = np.ascontiguousarray(np.asarray(x, dtype=np.float32))
    ei = np.asarray(edge_index)
    src_e = ei[0].astype(np.int64)
    dst_e = ei[1].astype(np.int64)

    npad = _cdiv(n, P * N_CORES) * P * N_CORES
    nb = npad // P
    nbc = nb // N_CORES

    # degrees include self loops
    deg = (np.bincount(dst_e, minlength=n) + 1).astype(np.float32)
    dinv = (1.0 / np.sqrt(deg)).astype(np.float32)
    d2 = np.zeros(npad, np.float32)  # dinv[dst], applied at the flush
    d2[:n] = dinv

    # dinv[src] folded into the gathered rows host-side
    xpad = np.zeros((npad, C), BF16)
    xpad[:n] = (x * dinv[:, None]).astype(BF16)

    # source ranges (int16 gather-idx windows)
    split = npad // 2
    bases = (max(0, split - IDX_WIN), max(0, npad - IDX_WIN))
    rng_of = (src_e >= split).astype(np.int64)

    # sort edges by cell = (dst block, range); cells contiguous
    blk = dst_e >> 7
    cell = blk * 2 + rng_of
    order = np.argsort(cell, kind="stable")
    srcs = src_e[order]
    dsts = dst_e[order]
    counts = np.bincount(cell[order], minlength=nb * 2).reshape(nb, 2)
    cell_starts = np.zeros(nb * 2 + 1, np.int64)
    np.cumsum(counts.reshape(-1), out=cell_starts[1:])

    # per-cell spans, uniform across cores (SPMD program): max count + 2
    # (>=2 pad slots so the call-final-negative swap always has a target)
    span = counts.reshape(N_CORES, nbc, 2).max(axis=0) + 2  # [nbc, 2]

    chunk_sizes = [min(CHUNK, nbc - i) for i in range(0, nbc, CHUNK)]

    # uniform section layouts: cells packed contiguously; tiles may
    # straddle two cells (one scatter matmul per covered cell)
    sections = []
    b0 = 0
    for cs in chunk_sizes:
        for r in (0, 1):
            offs = np.zeros(cs + 1, np.int64)
            np.cumsum(span[b0 : b0 + cs, r], out=offs[1:])
            tiles = _cdiv(int(offs[cs]), P)
            covers = []  # per tile: [(local block, soh col), ...]
            col = 0
            for j in range(tiles):
                lo, hi = j * P, (j + 1) * P
                cov = []
                for i in range(cs):
                    if offs[i] < hi and offs[i + 1] > lo:
                        cov.append((i, col))
                        col += 1
                covers.append(cov)
            sections.append(
                dict(b0=b0, cs=cs, r=r, offs=offs, tiles=tiles, ncols=col,
                     covers=covers)
            )
        b0 += cs
    n_tiles = sum(s["tiles"] for s in sections)
    n_cols = sum(s["ncols"] for s in sections)

    wt = np.ascontiguousarray(np.asarray(weight, np.float32).T)
    wiht = np.ascontiguousarray(np.asarray(w_ih, np.float32).T)
    bsum = (
        (np.asarray(b_ih, np.float32) + np.asarray(b_hh, np.float32))
        .reshape(4, C)
        .T.copy()
    )

    lane = np.arange(P, dtype=np.float32)[None, :]

    def wrap16(a):
        w = np.ascontiguousarray(a.reshape(-1, 16).T.astype(np.int16))
        return np.tile(w, (8, 1))  # [128, len/16]

    in_maps = []
    for m in range(N_CORES):
        gidx_cols = []
        soh_cols = []
        for sec in sections:
            b0s, cs, r, offs = sec["b0"], sec["cs"], sec["r"], sec["offs"]
            slots = sec["tiles"] * P
            sec_idx = np.zeros(slots, np.int32)  # pad idx 0 = valid row
            sec_dstl = np.full(slots, 200.0, np.float32)  # pad: no lane
            for i in range(cs):
                g = (m * nbc + b0s + i) * 2 + r
                cnt = int(cell_starts[g + 1] - cell_starts[g])
                sl = slice(cell_starts[g], cell_starts[g + 1])
                o = int(offs[i])
                sec_idx[o : o + cnt] = (srcs[sl] - bases[r]).astype(np.int32)
                sec_dstl[o : o + cnt] = (dsts[sl] & (P - 1)).astype(np.float32)
            # the gather ucode DROPS a trailing negative index: every
            # call-final slot must be >= 0 - swap offenders with a pad
            # slot of the same cell
            sec_tiles = sec["tiles"]
            call_ts = [CALL_T] * (sec_tiles // CALL_T)
            if sec_tiles % CALL_T:
                call_ts.append(sec_tiles % CALL_T)
            ends = np.cumsum(np.array(call_ts)) * P - 1
            end_set = set(int(e) for e in ends)
            for s in ends:
                s = int(s)
                if sec_idx[s] >= 0:
                    continue
                i = int(np.searchsorted(offs, s, side="right")) - 1
                g = (m * nbc + b0s + i) * 2 + r
                cnt = int(cell_starts[g + 1] - cell_starts[g])
                for p in range(int(offs[i]) + cnt, int(offs[i + 1])):
                    if p not in end_set:
                        sec_idx[s], sec_idx[p] = sec_idx[p], sec_idx[s]
                        sec_dstl[s], sec_dstl[p] = sec_dstl[p], sec_dstl[s]
                        break
                else:
                    raise RuntimeError("no swap slot for call-final pad")
            # gidx: wrapped int16, per call
            c0 = 0
            for ct in call_ts:
                gidx_cols.append(wrap16(sec_idx[c0 * P : (c0 + ct) * P]))
                c0 += ct
            # one-hot scatter columns, fp8: one per (tile, covered cell)
            slot_ids = np.arange(slots)
            for j, cov in enumerate(sec["covers"]):
                d_tile = sec_dstl[j * P : (j + 1) * P]
                s_ids = slot_ids[j * P : (j + 1) * P]
                for i, _col in cov:
                    belong = (s_ids >= offs[i]) & (s_ids < offs[i + 1])
                    oh = (d_tile[:, None] == lane) & belong[:, None]
                    soh_cols.append(oh)
        gidx = np.concatenate(gidx_cols, axis=1)
        gsoh = np.concatenate(soh_cols, axis=1).astype(FP8)

        lo_b = m * nbc
        d2_c = d2[lo_b * P : (lo_b + nbc) * P].reshape(nbc, P).T.copy()
        in_maps.append(
            dict(
                xsrc=xpad,
                xself=np.ascontiguousarray(xpad[lo_b * P : (lo_b + nbc) * P]),
                gidx=np.ascontiguousarray(gidx),
                gsoh=np.ascontiguousarray(gsoh),
                d2=d2_c,
                wt=wt,
                wiht=wiht,
                bias=bsum,
            )
        )
    meta = (sections, nbc, chunk_sizes, bases, npad, n_tiles, n_cols)
    return in_maps, meta


def build_program(meta, reps=1, variant="full", nq=NQ):
    import concourse.bacc as bacc
    import concourse.mybir as mybir
    import concourse.tile as tile
    from concourse.masks import make_identity

    sections, nbc, chunk_sizes, bases, npad, n_tiles, n_cols = meta
    f32 = mybir.dt.float32
    bf16 = mybir.dt.bfloat16
    fp8 = mybir.dt.float8e4
    i16 = mybir.dt.int16
    AF = mybir.ActivationFunctionType

    idx_w = n_tiles * P // 16  # gidx free dim (int16 cols)

    nc = bacc.Bacc("TRN2", num_swdge_queues=nq)
    xsrc = nc.declare_dram_parameter("xsrc", [npad, C], bf16, isOutput=False)
    xself = nc.declare_dram_parameter("xself", [nbc * P, C], bf16, isOutput=False)
    gidx = nc.declare_dram_parameter("gidx", [P, idx_w], i16, isOutput=False)
    gsoh = nc.declare_dram_parameter("gsoh", [P, n_cols * P], fp8, isOutput=False)
    d2 = nc.declare_dram_parameter("d2", [P, nbc], f32, isOutput=False)
    wt = nc.declare_dram_parameter("wt", [P, P], f32, isOutput=False)
    wiht = nc.declare_dram_parameter("wiht", [P, 4 * C], f32, isOutput=False)
    bias = nc.declare_dram_parameter("bias", [P, 4], f32, isOutput=False)
    out = nc.declare_dram_parameter("out", [nbc * P, C], f32, isOutput=True)

    with tile.TileContext(nc) as tc:
        with (
            tc.tile_pool(name="const", bufs=1) as constp,
            tc.tile_pool(name="stag", bufs=4) as stagp,
            tc.tile_pool(name="meta", bufs=4) as metap,
            tc.tile_pool(name="work", bufs=3) as workp,
            tc.tile_pool(name="selfx", bufs=8) as selfp,
            tc.tile_pool(name="psA", bufs=CHUNK, space="PSUM") as psA,
            tc.tile_pool(name="psB", bufs=1, space="PSUM") as psB,
            tc.tile_pool(name="osb", bufs=3) as osbp,
        ):
            wt_sb = constp.tile([P, P], f32, tag="wt")
            nc.sync.dma_start(out=wt_sb[:], in_=wt[:])
            wiht_sb = constp.tile([P, 4 * C], f32, tag="wiht")
            nc.sync.dma_start(out=wiht_sb[:], in_=wiht[:])
            bias_sb = constp.tile([P, 4], f32, tag="bias")
            nc.sync.dma_start(out=bias_sb[:], in_=bias[:])
            d2_sb = constp.tile([P, nbc], f32, tag="d2")
            nc.sync.dma_start(out=d2_sb[:], in_=d2[:])
            ident = constp.tile([P, P], f32, tag="ident")
            make_identity(nc, ident[:])
            identb = constp.tile([P, P], bf16, tag="identb")
            make_identity(nc, identb[:])

            # --- LSTM single step -> evolved weight w_new ---
            gate_sb = {}
            for g, func, bcol in ((0, AF.Sigmoid, 0), (2, AF.Tanh, 2), (3, AF.Sigmoid, 3)):
                ps = psB.tile([P, P], f32, tag="psb")
                nc.tensor.matmul(
                    out=ps[:],
                    lhsT=wiht_sb[:, g * P : (g + 1) * P],
                    rhs=wt_sb[:],
                    start=True,
                    stop=True,
                )
                sb = constp.tile([P, P], f32, tag=f"gate{g}")
                nc.scalar.activation(
                    out=sb[:], in_=ps[:], func=func, bias=bias_sb[:, bcol : bcol + 1]
                )
                gate_sb[g] = sb
            cT = constp.tile([P, P], f32, tag="cT")
            nc.vector.tensor_mul(out=cT[:], in0=gate_sb[0][:], in1=gate_sb[2][:])
            tcT = constp.tile([P, P], f32, tag="tcT")
            nc.scalar.activation(out=tcT[:], in_=cT[:], func=AF.Tanh)
            wnT = constp.tile([P, P], f32, tag="wnT")
            nc.vector.tensor_mul(out=wnT[:], in0=gate_sb[3][:], in1=tcT[:])
            wn_ps = psB.tile([P, P], f32, tag="psb")
            nc.tensor.transpose(out=wn_ps[:], in_=wnT[:], identity=ident[:])
            wn_sb = constp.tile([P, P], bf16, tag="wn")
            nc.vector.tensor_copy(out=wn_sb[:], in_=wn_ps[:])

            # --- main: chunks of blocks; self pass + 2 gather sections ---
            do_pe = variant in ("full", "noscat", "deadsoh")
            do_soh = variant in ("full", "gatherdve", "deadsoh")
            use_soh = variant == "full"
            do_gather = variant != "nogather"

            def emit_main(_iv=None):
              s_tile = 0  # global edge-tile cursor
              s_col = 0  # global one-hot column cursor
              n_call = 0  # gather call counter (queue round-robin)
              ci = 0
              b0 = 0
              for cs in chunk_sizes:
                if do_pe:
                    aggs = [
                        psA.tile([P, P], f32, name=f"agg{i}", tag="agg")
                        for i in range(cs)
                    ]
                    # self-loop pass: aggT[b] = (dinv*x_block)^T; the
                    # second dinv factor is applied at the flush
                    for i in range(cs):
                        b = b0 + i
                        xs = selfp.tile([P, P], bf16, tag="xself")
                        nc.sync.dma_start(
                            out=xs[:], in_=xself[b * P : (b + 1) * P, :]
                        )
                        nc.tensor.matmul(
                            out=aggs[i][:], lhsT=xs[:], rhs=identb[:],
                            start=True, stop=False,
                        )
                for r in range(2):
                    sec = sections[2 * ci + r]
                    sec_tiles = sec["tiles"]
                    idx_t = metap.tile([P, sec_tiles * 8], i16, tag="idx")
                    nc.sync.dma_start(
                        out=idx_t[:],
                        in_=gidx[:, s_tile * 8 : (s_tile + sec_tiles) * 8],
                    )
                    stag = stagp.tile([P, sec_tiles * P], bf16, tag="stag")
                    for c0 in range(0, sec_tiles, CALL_T):
                        if not do_gather:
                            break
                        ct = min(CALL_T, sec_tiles - c0)
                        nc.gpsimd.dma_gather(
                            out_ap=stag[:, c0 * P : (c0 + ct) * P].rearrange(
                                "p (t c) -> p t c", t=ct
                            ),
                            in_ap=xsrc[bases[r] :, :],
                            idxs_ap=idx_t[:, c0 * 8 : (c0 + ct) * 8],
                            num_idxs=ct * P,
                            num_idxs_reg=ct * P,
                            elem_size=P,
                            queue_num=n_call % nq,
                        )
                        n_call += 1
                    if not (do_pe or do_soh):
                        s_tile += sec_tiles
                        continue
                    if do_soh:
                        # host-precomputed one-hot scatter tiles, streamed
                        # on the ACT HWDGE ring (sync ring would head-of-
                        # line-block the idx loads; on-device DVE build
                        # would starve SWDGE desc-gen via the shared port)
                        s_oh = workp.tile([P, sec["ncols"] * P], fp8, tag="soh")
                        nc.scalar.dma_start(
                            out=s_oh[:],
                            in_=gsoh[:, s_col * P : (s_col + sec["ncols"]) * P],
                        )
                    if do_pe:
                        # stop flag goes on each block's last scatter matmul
                        last_of = {}
                        if r == 1:
                            for j, cov in enumerate(sec["covers"]):
                                for i, col in cov:
                                    last_of[i] = (j, col)
                        for j, cov in enumerate(sec["covers"]):
                            for i, col in cov:
                                nc.tensor.matmul(
                                    out=aggs[i][:],
                                    lhsT=stag[:, j * P : (j + 1) * P],
                                    rhs=s_oh[:, col * P : (col + 1) * P]
                                    if use_soh
                                    else identb[:],
                                    start=False,
                                    stop=(r == 1 and last_of.get(i) == (j, col)),
                                )
                    s_tile += sec_tiles
                    s_col += sec["ncols"]
                # flush chunk
                for i in range(cs if do_pe else 0):
                    b = b0 + i
                    agg_sb = osbp.tile([P, P], bf16, tag="aggsb")
                    nc.scalar.activation(out=agg_sb[:], in_=aggs[i][:], func=AF.Copy)
                    y_ps = psB.tile([P, P], f32, tag="psb")
                    nc.tensor.matmul(
                        out=y_ps[:], lhsT=agg_sb[:], rhs=wn_sb[:],
                        start=True, stop=True,
                    )
                    y_sb = osbp.tile([P, P], f32, tag="ysb")
                    nc.scalar.activation(
                        out=y_sb[:], in_=y_ps[:], func=AF.Copy,
                        scale=d2_sb[:, b : b + 1],
                    )
                    nc.sync.dma_start(
                        out=out[b * P : (b + 1) * P, :], in_=y_sb[:]
                    )
                b0 += cs
                ci += 1

            if reps > 1:
                with tc.For_i(0, reps, 1):
                    emit_main()
            else:
                emit_main()

    nc.finalize()
    return nc


def kernel(**inputs) -> np.ndarray:
    from concourse.bass_utils import run_bass_kernel_spmd

    x = inputs["x"]
    n = x.shape[0]
    in_maps, meta = prep_inputs(
        x,
        inputs["edge_index"],
        inputs["weight"],
        inputs["w_ih"],
        inputs["b_ih"],
        inputs["b_hh"],
        n=n,
    )
    nc = build_program(meta)
    res = run_bass_kernel_spmd(nc, in_maps, list(range(N_CORES)))
    full = np.concatenate([r["out"] for r in res.results], axis=0)
    return np.ascontiguousarray(full[:n])


# revision 63
# speedup vs baseline: 1.4615x; 1.0333x over previous
"""EvolveGCN-O forward pass on 8 Trainium2 NeuronCores (Bass/Tile).

Math (reference):
    w_new = LSTM-evolve(weight; w_ih, b_ih+b_hh)          # [C, C]
    out   = D^-1/2 (A + I) D^-1/2  X  w_new               # [N, C]

Device strategy (edges + scatter targets sharded per the hint):
  * Destination nodes padded to NPAD; each core owns nbc consecutive
    128-node blocks, processed in chunks of 7 (7 PSUM banks accumulate
    7 blocks; the 8th bank is scratch).
  * dinv[src] is folded into the gathered rows host-side (xsrc holds
    dinv[i]*x[i] in bf16); dinv[dst] is applied by the per-block flush
    (scalar-engine PSUM->SBUF copy with per-partition scale). No
    per-edge scaling on device.
  * Edges sorted by (dst block, src range); the dma_gather index is a
    SIGNED int16 row offset, so 2 source ranges cover N=100k. Per
    (chunk, range) section, cells (one per block) are packed
    contiguously with span = max-over-cores count + 2 (uniform across
    cores - the program is SPMD). Tiles straddling two cells scatter
    into both blocks via two matmuls.
  * gpsimd.dma_gather stages 256B bf16 rows; desc-gen runs on the Q7
    pair 2q/2q+1, so calls round-robin over 4 SWDGE queues to
    parallelize descriptor generation (the real bottleneck).
  * The dst one-hot scatter tiles are PRECOMPUTED on host (fp8) and
    streamed via the ACT HWDGE ring: building them on device (DVE
    is_equal) stalls SWDGE desc-gen via the shared SBUF port pair, and
    streaming them on the sync ring head-of-line-blocks the idx loads.
  * PE accumulates aggT[ch,lane] += stag_tile^T @ onehot in PSUM fp32;
    per block: Y = aggT^T @ w_new, scaled by dinv[dst], DMA out.
  * w_new computed on-device (3 matmuls + activations), redundantly per
    core. No collectives: block ownership makes outputs disjoint.
"""
import os as _os
import sys

for _p in ("/opt/trn_rl_repo", "/root/.axon_site/_ro/trn_rl_repo"):
    if _p not in sys.path:
        sys.path.append(_p)

import ml_dtypes
import numpy as np

BF16 = ml_dtypes.bfloat16
FP8 = ml_dtypes.float8_e4m3

N, C, E = 100000, 128, 1600000  # problem shape (hardcoded per spec)
P = 128
N_CORES = 8
CHUNK = 7  # blocks per PSUM-resident chunk (7 psA banks + psB = 8)
IDX_WIN = 32768  # int16 signed reach below/above base
CALL_T = int(_os.environ.get("CALL_T", "8"))  # edge tiles per dma_gather call
NQ = 4  # SWDGE queues: gather desc-gen runs on Q7 core pair 2q/2q+1


def _cdiv(a, b):
    return -(-a // b)


def prep_inputs(x, edge_index, weight, w_ih, b_ih, b_hh, n=N):
    """Host-side sharding/index prep. Returns (in_maps, meta)."""
    x = np.ascontiguousarray(np.asarray(x, dtype=np.float32))
    ei = np.asarray(edge_index)
    src_e = ei[0].astype(np.int64)
    dst_e = ei[1].astype(np.int64)

    npad = _cdiv(n, P * N_CORES) * P * N_CORES
    nb = npad // P
    nbc = nb // N_CORES

    # degrees include self loops
    deg = (np.bincount(dst_e, minlength=n) + 1).astype(np.float32)
    dinv = (1.0 / np.sqrt(deg)).astype(np.float32)
    d2 = np.zeros(npad, np.float32)  # dinv[dst], applied at the flush
    d2[:n] = dinv

    # dinv[src] folded into the gathered rows host-side
    xpad = np.zeros((npad, C), BF16)
    xpad[:n] = (x * dinv[:, None]).astype(BF16)

    # source ranges (int16 gather-idx windows)
    split = npad // 2
    bases = (max(0, split - IDX_WIN), max(0, npad - IDX_WIN))
    rng_of = (src_e >= split).astype(np.int64)

    # sort edges by cell = (dst block, range); cells contiguous
    blk = dst_e >> 7
    cell = blk * 2 + rng_of
    order = np.argsort(cell, kind="stable")
    srcs = src_e[order]
    dsts = dst_e[order]
    counts = np.bincount(cell[order], minlength=nb * 2).reshape(nb, 2)
    cell_starts = np.zeros(nb * 2 + 1, np.int64)
    np.cumsum(counts.reshape(-1), out=cell_starts[1:])

    # per-cell spans, uniform across cores (SPMD program): max count + 2
    # (>=2 pad slots so the call-final-negative swap always has a target)
    span = counts.reshape(N_CORES, nbc, 2).max(axis=0) + 2  # [nbc, 2]

    chunk_sizes = [min(CHUNK, nbc - i) for i in range(0, nbc, CHUNK)]

    # uniform section layouts: cells packed contiguously; tiles may
    # straddle two cells (one scatter matmul per covered cell)
    sections = []
    b0 = 0
    for cs in chunk_sizes:
        for r in (0, 1):
            offs = np.zeros(cs + 1, np.int64)
            np.cumsum(span[b0 : b0 + cs, r], out=offs[1:])
            tiles = _cdiv(int(offs[cs]), P)
            covers = []  # per tile: [(local block, soh col), ...]
            col = 0
            for j in range(tiles):
                lo, hi = j * P, (j + 1) * P
                cov = []
                for i in range(cs):
                    if offs[i] < hi and offs[i + 1] > lo:
                        cov.append((i, col))
                        col += 1
                covers.append(cov)
            sections.append(
                dict(b0=b0, cs=cs, r=r, offs=offs, tiles=tiles, ncols=col,
                     covers=covers)
            )
        b0 += cs
    n_tiles = sum(s["tiles"] for s in sections)
    n_cols = sum(s["ncols"] for s in sections)

    wt = np.ascontiguousarray(np.asarray(weight, np.float32).T)
    wiht = np.ascontiguousarray(np.asarray(w_ih, np.float32).T)
    bsum = (
        (np.asarray(b_ih, np.float32) + np.asarray(b_hh, np.float32))
        .reshape(4, C)
        .T.copy()
    )

    lane = np.arange(P, dtype=np.float32)[None, :]

    def wrap16(a):
        w = np.ascontiguousarray(a.reshape(-1, 16).T.astype(np.int16))
        return np.tile(w, (8, 1))  # [128, len/16]

    in_maps = []
    for m in range(N_CORES):
        gidx_cols = []
        soh_cols = []
        for sec in sections:
            b0s, cs, r, offs = sec["b0"], sec["cs"], sec["r"], sec["offs"]
            slots = sec["tiles"] * P
            sec_idx = np.zeros(slots, np.int32)  # pad idx 0 = valid row
            sec_dstl = np.full(slots, 200.0, np.float32)  # pad: no lane
            for i in range(cs):
                g = (m * nbc + b0s + i) * 2 + r
                cnt = int(cell_starts[g + 1] - cell_starts[g])
                sl = slice(cell_starts[g], cell_starts[g + 1])
                o = int(offs[i])
                sec_idx[o : o + cnt] = (srcs[sl] - bases[r]).astype(np.int32)
                sec_dstl[o : o + cnt] = (dsts[sl] & (P - 1)).astype(np.float32)
            # the gather ucode DROPS a trailing negative index: every
            # call-final slot must be >= 0 - swap offenders with a pad
            # slot of the same cell
            sec_tiles = sec["tiles"]
            call_ts = [CALL_T] * (sec_tiles // CALL_T)
            if sec_tiles % CALL_T:
                call_ts.append(sec_tiles % CALL_T)
            ends = np.cumsum(np.array(call_ts)) * P - 1
            end_set = set(int(e) for e in ends)
            for s in ends:
                s = int(s)
                if sec_idx[s] >= 0:
                    continue
                i = int(np.searchsorted(offs, s, side="right")) - 1
                g = (m * nbc + b0s + i) * 2 + r
                cnt = int(cell_starts[g + 1] - cell_starts[g])
                for p in range(int(offs[i]) + cnt, int(offs[i + 1])):
                    if p not in end_set:
                        sec_idx[s], sec_idx[p] = sec_idx[p], sec_idx[s]
                        sec_dstl[s], sec_dstl[p] = sec_dstl[p], sec_dstl[s]
                        break
                else:
                    raise RuntimeError("no swap slot for call-final pad")
            # gidx: wrapped int16, per call
            c0 = 0
            for ct in call_ts:
                gidx_cols.append(wrap16(sec_idx[c0 * P : (c0 + ct) * P]))
                c0 += ct
            # one-hot scatter columns, fp8: one per (tile, covered cell)
            slot_ids = np.arange(slots)
            for j, cov in enumerate(sec["covers"]):
                d_tile = sec_dstl[j * P : (j + 1) * P]
                s_ids = slot_ids[j * P : (j + 1) * P]
                for i, _col in cov:
                    belong = (s_ids >= offs[i]) & (s_ids < offs[i + 1])
                    oh = (d_tile[:, None] == lane) & belong[:, None]
                    soh_cols.append(oh)
        gidx = np.concatenate(gidx_cols, axis=1)
        gsoh = np.concatenate(soh_cols, axis=1).astype(FP8)

        lo_b = m * nbc
        d2_c = d2[lo_b * P : (lo_b + nbc) * P].reshape(nbc, P).T.copy()
        in_maps.append(
            dict(
                xsrc=xpad,
                xself=np.ascontiguousarray(xpad[lo_b * P : (lo_b + nbc) * P]),
                gidx=np.ascontiguousarray(gidx),
                gsoh=np.ascontiguousarray(gsoh),
                d2=d2_c,
                wt=wt,
                wiht=wiht,
                bias=bsum,
            )
        )
    meta = (sections, nbc, chunk_sizes, bases, npad, n_tiles, n_cols)
    return in_maps, meta


def build_program(meta, reps=1, variant="full", nq=NQ):
    import concourse.bacc as bacc
    import concourse.mybir as mybir
    import concourse.tile as tile
    from concourse.masks import make_identity

    sections, nbc, chunk_sizes, bases, npad, n_tiles, n_cols = meta
    f32 = mybir.dt.float32
    bf16 = mybir.dt.bfloat16
    fp8 = mybir.dt.float8e4
    i16 = mybir.dt.int16
    AF = mybir.ActivationFunctionType

    idx_w = n_tiles * P // 16  # gidx free dim (int16 cols)

    nc = bacc.Bacc("TRN2", num_swdge_queues=nq)
    xsrc = nc.declare_dram_parameter("xsrc", [npad, C], bf16, isOutput=False)
    xself = nc.declare_dram_parameter("xself", [nbc * P, C], bf16, isOutput=False)
    gidx = nc.declare_dram_parameter("gidx", [P, idx_w], i16, isOutput=False)
    gsoh = nc.declare_dram_parameter("gsoh", [P, n_cols * P], fp8, isOutput=False)
    d2 = nc.declare_dram_parameter("d2", [P, nbc], f32, isOutput=False)
    wt = nc.declare_dram_parameter("wt", [P, P], f32, isOutput=False)
    wiht = nc.declare_dram_parameter("wiht", [P, 4 * C], f32, isOutput=False)
    bias = nc.declare_dram_parameter("bias", [P, 4], f32, isOutput=False)
    out = nc.declare_dram_parameter("out", [nbc * P, C], f32, isOutput=True)

    with tile.TileContext(nc) as tc:
        with (
            tc.tile_pool(name="const", bufs=1) as constp,
            tc.tile_pool(name="stag", bufs=4) as stagp,
            tc.tile_pool(name="meta", bufs=4) as metap,
            tc.tile_pool(name="work", bufs=3) as workp,
            tc.tile_pool(name="selfx", bufs=8) as selfp,
            tc.tile_pool(name="psA", bufs=CHUNK, space="PSUM") as psA,
            tc.tile_pool(name="psB", bufs=1, space="PSUM") as psB,
            tc.tile_pool(name="osb", bufs=3) as osbp,
        ):
            wt_sb = constp.tile([P, P], f32, tag="wt")
            nc.sync.dma_start(out=wt_sb[:], in_=wt[:])
            wiht_sb = constp.tile([P, 4 * C], f32, tag="wiht")
            nc.sync.dma_start(out=wiht_sb[:], in_=wiht[:])
            bias_sb = constp.tile([P, 4], f32, tag="bias")
            nc.sync.dma_start(out=bias_sb[:], in_=bias[:])
            d2_sb = constp.tile([P, nbc], f32, tag="d2")
            nc.sync.dma_start(out=d2_sb[:], in_=d2[:])
            ident = constp.tile([P, P], f32, tag="ident")
            make_identity(nc, ident[:])
            identb = constp.tile([P, P], bf16, tag="identb")
            make_identity(nc, identb[:])

            # --- LSTM single step -> evolved weight w_new ---
            gate_sb = {}
            for g, func, bcol in ((0, AF.Sigmoid, 0), (2, AF.Tanh, 2), (3, AF.Sigmoid, 3)):
                ps = psB.tile([P, P], f32, tag="psb")
                nc.tensor.matmul(
                    out=ps[:],
                    lhsT=wiht_sb[:, g * P : (g + 1) * P],
                    rhs=wt_sb[:],
                    start=True,
                    stop=True,
                )
                sb = constp.tile([P, P], f32, tag=f"gate{g}")
                nc.scalar.activation(
                    out=sb[:], in_=ps[:], func=func, bias=bias_sb[:, bcol : bcol + 1]
                )
                gate_sb[g] = sb
            cT = constp.tile([P, P], f32, tag="cT")
            nc.vector.tensor_mul(out=cT[:], in0=gate_sb[0][:], in1=gate_sb[2][:])
            tcT = constp.tile([P, P], f32, tag="tcT")
            nc.scalar.activation(out=tcT[:], in_=cT[:], func=AF.Tanh)
            wnT = constp.tile([P, P], f32, tag="wnT")
            nc.vector.tensor_mul(out=wnT[:], in0=gate_sb[3][:], in1=tcT[:])
            wn_ps = psB.tile([P, P], f32, tag="psb")
            nc.tensor.transpose(out=wn_ps[:], in_=wnT[:], identity=ident[:])
            wn_sb = constp.tile([P, P], bf16, tag="wn")
            nc.vector.tensor_copy(out=wn_sb[:], in_=wn_ps[:])

            # --- main: chunks of blocks; self pass + 2 gather sections ---
            do_pe = variant in ("full", "noscat", "deadsoh")
            do_soh = variant in ("full", "gatherdve", "deadsoh")
            use_soh = variant == "full"
            do_gather = variant != "nogather"

            # greedy min-load queue assignment: with ~8 calls/section a
            # plain n_call%nq pins every partial (light) tail call to the
            # same queue, and the other queues' extra rows set the makespan
            call_sizes = []
            for sec in sections:
                st = sec["tiles"]
                for c0 in range(0, st, CALL_T):
                    call_sizes.append(min(CALL_T, st - c0))
            qload = [0] * nq
            call_q = []
            for sz in call_sizes:
                q = min(range(nq), key=lambda k: qload[k])
                call_q.append(q)
                qload[q] += sz

            def emit_main(_iv=None):
              s_tile = 0  # global edge-tile cursor
              s_col = 0  # global one-hot column cursor
              n_call = 0  # gather call counter (queue round-robin)
              ci = 0
              b0 = 0
              for cs in chunk_sizes:
                if do_pe:
                    aggs = [
                        psA.tile([P, P], f32, name=f"agg{i}", tag="agg")
                        for i in range(cs)
                    ]
                    # self-loop pass: aggT[b] = (dinv*x_block)^T; the
                    # second dinv factor is applied at the flush
                    for i in range(cs):
                        b = b0 + i
                        xs = selfp.tile([P, P], bf16, tag="xself")
                        nc.sync.dma_start(
                            out=xs[:], in_=xself[b * P : (b + 1) * P, :]
                        )
                        nc.tensor.matmul(
                            out=aggs[i][:], lhsT=xs[:], rhs=identb[:],
                            start=True, stop=False,
                        )
                for r in range(2):
                    sec = sections[2 * ci + r]
                    sec_tiles = sec["tiles"]
                    idx_t = metap.tile([P, sec_tiles * 8], i16, tag="idx")
                    nc.sync.dma_start(
                        out=idx_t[:],
                        in_=gidx[:, s_tile * 8 : (s_tile + sec_tiles) * 8],
                    )
                    stag = stagp.tile([P, sec_tiles * P], bf16, tag="stag")
                    for c0 in range(0, sec_tiles, CALL_T):
                        if not do_gather:
                            break
                        ct = min(CALL_T, sec_tiles - c0)
                        nc.gpsimd.dma_gather(
                            out_ap=stag[:, c0 * P : (c0 + ct) * P].rearrange(
                                "p (t c) -> p t c", t=ct
                            ),
                            in_ap=xsrc[bases[r] :, :],
                            idxs_ap=idx_t[:, c0 * 8 : (c0 + ct) * 8],
                            num_idxs=ct * P,
                            num_idxs_reg=ct * P,
                            elem_size=P,
                            queue_num=call_q[n_call],
                        )
                        n_call += 1
                    if not (do_pe or do_soh):
                        s_tile += sec_tiles
                        continue
                    if do_soh:
                        # host-precomputed one-hot scatter tiles, streamed
                        # on the ACT HWDGE ring (sync ring would head-of-
                        # line-block the idx loads; on-device DVE build
                        # would starve SWDGE desc-gen via the shared port)
                        s_oh = workp.tile([P, sec["ncols"] * P], fp8, tag="soh")
                        nc.scalar.dma_start(
                            out=s_oh[:],
                            in_=gsoh[:, s_col * P : (s_col + sec["ncols"]) * P],
                        )
                    if do_pe:
                        # stop flag goes on each block's last scatter matmul
                        last_of = {}
                        if r == 1:
                            for j, cov in enumerate(sec["covers"]):
                                for i, col in cov:
                                    last_of[i] = (j, col)
                        for j, cov in enumerate(sec["covers"]):
                            for i, col in cov:
                                nc.tensor.matmul(
                                    out=aggs[i][:],
                                    lhsT=stag[:, j * P : (j + 1) * P],
                                    rhs=s_oh[:, col * P : (col + 1) * P]
                                    if use_soh
                                    else identb[:],
                                    start=False,
                                    stop=(r == 1 and last_of.get(i) == (j, col)),
                                )
                    s_tile += sec_tiles
                    s_col += sec["ncols"]
                # flush chunk
                for i in range(cs if do_pe else 0):
                    b = b0 + i
                    agg_sb = osbp.tile([P, P], bf16, tag="aggsb")
                    nc.scalar.activation(out=agg_sb[:], in_=aggs[i][:], func=AF.Copy)
                    y_ps = psB.tile([P, P], f32, tag="psb")
                    nc.tensor.matmul(
                        out=y_ps[:], lhsT=agg_sb[:], rhs=wn_sb[:],
                        start=True, stop=True,
                    )
                    y_sb = osbp.tile([P, P], f32, tag="ysb")
                    nc.scalar.activation(
                        out=y_sb[:], in_=y_ps[:], func=AF.Copy,
                        scale=d2_sb[:, b : b + 1],
                    )
                    nc.sync.dma_start(
                        out=out[b * P : (b + 1) * P, :], in_=y_sb[:]
                    )
                b0 += cs
                ci += 1

            if reps > 1:
                with tc.For_i(0, reps, 1):
                    emit_main()
            else:
                emit_main()

    nc.finalize()
    return nc


def kernel(**inputs) -> np.ndarray:
    from concourse.bass_utils import run_bass_kernel_spmd

    x = inputs["x"]
    n = x.shape[0]
    in_maps, meta = prep_inputs(
        x,
        inputs["edge_index"],
        inputs["weight"],
        inputs["w_ih"],
        inputs["b_ih"],
        inputs["b_hh"],
        n=n,
    )
    nc = build_program(meta)
    res = run_bass_kernel_spmd(nc, in_maps, list(range(N_CORES)))
    full = np.concatenate([r["out"] for r in res.results], axis=0)
    return np.ascontiguousarray(full[:n])


# revision 65
# speedup vs baseline: 1.5771x; 1.0791x over previous
"""EvolveGCN-O forward pass on 8 Trainium2 NeuronCores (Bass/Tile).

Math (reference):
    w_new = LSTM-evolve(weight; w_ih, b_ih+b_hh)          # [C, C]
    out   = D^-1/2 (A + I) D^-1/2  X  w_new               # [N, C]

Device strategy (edges + scatter targets sharded per the hint):
  * Destination nodes padded to NPAD; each core owns nbc consecutive
    128-node blocks, processed in chunks of 7 (7 PSUM banks accumulate
    7 blocks; the 8th bank is scratch).
  * dinv[src] is folded into the gathered rows host-side (xsrc holds
    dinv[i]*x[i] in bf16); dinv[dst] is applied by the per-block flush
    (scalar-engine PSUM->SBUF copy with per-partition scale). No
    per-edge scaling on device.
  * Edges sorted by (dst block, src range); the dma_gather index is a
    SIGNED int16 row offset, so 2 source ranges cover N=100k. Per
    (chunk, range) section, cells (one per block) are packed
    contiguously with span = max-over-cores count + 2 (uniform across
    cores - the program is SPMD). Tiles straddling two cells scatter
    into both blocks via two matmuls.
  * gpsimd.dma_gather stages 256B bf16 rows; desc-gen runs on the Q7
    pair 2q/2q+1, so calls round-robin over 4 SWDGE queues to
    parallelize descriptor generation (the real bottleneck).
  * The dst one-hot scatter tiles are PRECOMPUTED on host (fp8) and
    streamed via the ACT HWDGE ring: building them on device (DVE
    is_equal) stalls SWDGE desc-gen via the shared SBUF port pair, and
    streaming them on the sync ring head-of-line-blocks the idx loads.
  * PE accumulates aggT[ch,lane] += stag_tile^T @ onehot in PSUM fp32;
    per block: Y = aggT^T @ w_new, scaled by dinv[dst], DMA out.
  * w_new computed on-device (3 matmuls + activations), redundantly per
    core. No collectives: block ownership makes outputs disjoint.
"""
import os as _os
import sys

for _p in ("/opt/trn_rl_repo", "/root/.axon_site/_ro/trn_rl_repo"):
    if _p not in sys.path:
        sys.path.append(_p)

import ml_dtypes
import numpy as np

BF16 = ml_dtypes.bfloat16
FP8 = ml_dtypes.float8_e4m3

N, C, E = 100000, 128, 1600000  # problem shape (hardcoded per spec)
P = 128
N_CORES = 8
CHUNK = 7  # blocks per PSUM-resident chunk (7 psA banks + psB = 8)
IDX_WIN = 32768  # int16 signed reach below/above base
CALL_T = int(_os.environ.get("CALL_T", "8"))  # edge tiles per dma_gather call
NQ = 4  # SWDGE queues: gather desc-gen runs on Q7 core pair 2q/2q+1


def _cdiv(a, b):
    return -(-a // b)


def prep_inputs(x, edge_index, weight, w_ih, b_ih, b_hh, n=N):
    """Host-side sharding/index prep. Returns (in_maps, meta)."""
    x = np.ascontiguousarray(np.asarray(x, dtype=np.float32))
    ei = np.asarray(edge_index)
    src_e = ei[0].astype(np.int64)
    dst_e = ei[1].astype(np.int64)

    npad = _cdiv(n, P * N_CORES) * P * N_CORES
    nb = npad // P
    nbc = nb // N_CORES

    # degrees include self loops
    deg = (np.bincount(dst_e, minlength=n) + 1).astype(np.float32)
    dinv = (1.0 / np.sqrt(deg)).astype(np.float32)
    d2 = np.zeros(npad, np.float32)  # dinv[dst], applied at the flush
    d2[:n] = dinv

    # dinv[src] folded into the gathered rows host-side
    xpad = np.zeros((npad, C), BF16)
    xpad[:n] = (x * dinv[:, None]).astype(BF16)

    # source ranges (int16 gather-idx windows)
    split = npad // 2
    bases = (max(0, split - IDX_WIN), max(0, npad - IDX_WIN))
    rng_of = (src_e >= split).astype(np.int64)

    # sort edges by cell = (dst block, range); cells contiguous
    blk = dst_e >> 7
    cell = blk * 2 + rng_of
    order = np.argsort(cell, kind="stable")
    srcs = src_e[order]
    dsts = dst_e[order]
    counts = np.bincount(cell[order], minlength=nb * 2).reshape(nb, 2)
    cell_starts = np.zeros(nb * 2 + 1, np.int64)
    np.cumsum(counts.reshape(-1), out=cell_starts[1:])

    counts_c = counts.reshape(N_CORES, nbc, 2)  # [core, block, range]
    chunk_sizes = [min(CHUNK, nbc - i) for i in range(0, nbc, CHUNK)]

    # exact per-core packing: each core places its cells back to back
    # (offs_by_core); the SPMD program's per-tile scatter coverage is the
    # UNION over cores of blocks intersecting that tile (a zero one-hot
    # column makes a foreign matmul harmless for cores it doesn't apply to)
    sections = []
    b0 = 0
    for cs in chunk_sizes:
        for r in (0, 1):
            offs_by_core = []
            for m in range(N_CORES):
                offs = np.zeros(cs + 1, np.int64)
                np.cumsum(counts_c[m, b0 : b0 + cs, r], out=offs[1:])
                offs_by_core.append(offs)
            tiles = max(_cdiv(int(o[cs]), P) for o in offs_by_core)
            covers = []  # per tile: [(local block, soh col), ...]
            col = 0
            for j in range(tiles):
                lo, hi = j * P, (j + 1) * P
                cov = []
                for i in range(cs):
                    if any(
                        o[i] < hi and o[i + 1] > lo for o in offs_by_core
                    ):
                        cov.append((i, col))
                        col += 1
                covers.append(cov)
            sections.append(
                dict(b0=b0, cs=cs, r=r, offs_by_core=offs_by_core,
                     tiles=tiles, ncols=col, covers=covers)
            )
        b0 += cs
    n_tiles = sum(s["tiles"] for s in sections)
    n_cols = sum(s["ncols"] for s in sections)

    wt = np.ascontiguousarray(np.asarray(weight, np.float32).T)
    wiht = np.ascontiguousarray(np.asarray(w_ih, np.float32).T)
    bsum = (
        (np.asarray(b_ih, np.float32) + np.asarray(b_hh, np.float32))
        .reshape(4, C)
        .T.copy()
    )

    lane = np.arange(P, dtype=np.float32)[None, :]

    def wrap16(a):
        w = np.ascontiguousarray(a.reshape(-1, 16).T.astype(np.int16))
        return np.tile(w, (8, 1))  # [128, len/16]

    in_maps = []
    for m in range(N_CORES):
        gidx_cols = []
        soh_cols = []
        for sec in sections:
            b0s, cs, r = sec["b0"], sec["cs"], sec["r"]
            offs = sec["offs_by_core"][m]
            slots = sec["tiles"] * P
            sec_idx = np.zeros(slots, np.int32)  # pad idx 0 = valid row
            sec_dstl = np.full(slots, 200.0, np.float32)  # pad: no lane
            for i in range(cs):
                g = (m * nbc + b0s + i) * 2 + r
                cnt = int(cell_starts[g + 1] - cell_starts[g])
                sl = slice(cell_starts[g], cell_starts[g + 1])
                o = int(offs[i])
                sec_idx[o : o + cnt] = (srcs[sl] - bases[r]).astype(np.int32)
                sec_dstl[o : o + cnt] = (dsts[sl] & (P - 1)).astype(np.float32)
            # the gather ucode DROPS a trailing negative index: every
            # call-final slot must be >= 0 - swap offenders with any
            # non-negative slot of the same cell (same block -> the
            # one-hot columns, built after this, follow the swap)
            sec_tiles = sec["tiles"]
            call_ts = [CALL_T] * (sec_tiles // CALL_T)
            if sec_tiles % CALL_T:
                call_ts.append(sec_tiles % CALL_T)
            ends = np.cumsum(np.array(call_ts)) * P - 1
            end_set = set(int(e) for e in ends)
            for s in ends:
                s = int(s)
                if sec_idx[s] >= 0:
                    continue
                i = int(np.searchsorted(offs, s, side="right")) - 1
                for p in range(int(offs[i]), int(offs[i + 1])):
                    if p not in end_set and sec_idx[p] >= 0:
                        sec_idx[s], sec_idx[p] = sec_idx[p], sec_idx[s]
                        sec_dstl[s], sec_dstl[p] = sec_dstl[p], sec_dstl[s]
                        break
                else:
                    raise RuntimeError("no swap slot for call-final pad")
            # gidx: wrapped int16, per call
            c0 = 0
            for ct in call_ts:
                gidx_cols.append(wrap16(sec_idx[c0 * P : (c0 + ct) * P]))
                c0 += ct
            # one-hot scatter columns, fp8: one per (tile, covered cell)
            slot_ids = np.arange(slots)
            for j, cov in enumerate(sec["covers"]):
                d_tile = sec_dstl[j * P : (j + 1) * P]
                s_ids = slot_ids[j * P : (j + 1) * P]
                for i, _col in cov:
                    belong = (s_ids >= offs[i]) & (s_ids < offs[i + 1])
                    oh = (d_tile[:, None] == lane) & belong[:, None]
                    soh_cols.append(oh)
        gidx = np.concatenate(gidx_cols, axis=1)
        gsoh = np.concatenate(soh_cols, axis=1).astype(FP8)

        lo_b = m * nbc
        d2_c = d2[lo_b * P : (lo_b + nbc) * P].reshape(nbc, P).T.copy()
        in_maps.append(
            dict(
                xsrc=xpad,
                xself=np.ascontiguousarray(xpad[lo_b * P : (lo_b + nbc) * P]),
                gidx=np.ascontiguousarray(gidx),
                gsoh=np.ascontiguousarray(gsoh),
                d2=d2_c,
                wt=wt,
                wiht=wiht,
                bias=bsum,
            )
        )
    meta = (sections, nbc, chunk_sizes, bases, npad, n_tiles, n_cols)
    return in_maps, meta


def build_program(meta, reps=1, variant="full", nq=NQ):
    import concourse.bacc as bacc
    import concourse.mybir as mybir
    import concourse.tile as tile
    from concourse.masks import make_identity

    sections, nbc, chunk_sizes, bases, npad, n_tiles, n_cols = meta
    f32 = mybir.dt.float32
    bf16 = mybir.dt.bfloat16
    fp8 = mybir.dt.float8e4
    i16 = mybir.dt.int16
    AF = mybir.ActivationFunctionType

    idx_w = n_tiles * P // 16  # gidx free dim (int16 cols)

    nc = bacc.Bacc("TRN2", num_swdge_queues=nq)
    xsrc = nc.declare_dram_parameter("xsrc", [npad, C], bf16, isOutput=False)
    xself = nc.declare_dram_parameter("xself", [nbc * P, C], bf16, isOutput=False)
    gidx = nc.declare_dram_parameter("gidx", [P, idx_w], i16, isOutput=False)
    gsoh = nc.declare_dram_parameter("gsoh", [P, n_cols * P], fp8, isOutput=False)
    d2 = nc.declare_dram_parameter("d2", [P, nbc], f32, isOutput=False)
    wt = nc.declare_dram_parameter("wt", [P, P], f32, isOutput=False)
    wiht = nc.declare_dram_parameter("wiht", [P, 4 * C], f32, isOutput=False)
    bias = nc.declare_dram_parameter("bias", [P, 4], f32, isOutput=False)
    out = nc.declare_dram_parameter("out", [nbc * P, C], f32, isOutput=True)

    with tile.TileContext(nc) as tc:
        with (
            tc.tile_pool(name="const", bufs=1) as constp,
            tc.tile_pool(name="stag", bufs=4) as stagp,
            tc.tile_pool(name="meta", bufs=4) as metap,
            tc.tile_pool(name="work", bufs=3) as workp,
            tc.tile_pool(name="selfx", bufs=8) as selfp,
            tc.tile_pool(name="psA", bufs=CHUNK, space="PSUM") as psA,
            tc.tile_pool(name="psB", bufs=1, space="PSUM") as psB,
            tc.tile_pool(name="osb", bufs=3) as osbp,
        ):
            wt_sb = constp.tile([P, P], f32, tag="wt")
            nc.sync.dma_start(out=wt_sb[:], in_=wt[:])
            wiht_sb = constp.tile([P, 4 * C], f32, tag="wiht")
            nc.sync.dma_start(out=wiht_sb[:], in_=wiht[:])
            bias_sb = constp.tile([P, 4], f32, tag="bias")
            nc.sync.dma_start(out=bias_sb[:], in_=bias[:])
            d2_sb = constp.tile([P, nbc], f32, tag="d2")
            nc.sync.dma_start(out=d2_sb[:], in_=d2[:])
            ident = constp.tile([P, P], f32, tag="ident")
            make_identity(nc, ident[:])
            identb = constp.tile([P, P], bf16, tag="identb")
            make_identity(nc, identb[:])

            # --- LSTM single step -> evolved weight w_new ---
            gate_sb = {}
            for g, func, bcol in ((0, AF.Sigmoid, 0), (2, AF.Tanh, 2), (3, AF.Sigmoid, 3)):
                ps = psB.tile([P, P], f32, tag="psb")
                nc.tensor.matmul(
                    out=ps[:],
                    lhsT=wiht_sb[:, g * P : (g + 1) * P],
                    rhs=wt_sb[:],
                    start=True,
                    stop=True,
                )
                sb = constp.tile([P, P], f32, tag=f"gate{g}")
                nc.scalar.activation(
                    out=sb[:], in_=ps[:], func=func, bias=bias_sb[:, bcol : bcol + 1]
                )
                gate_sb[g] = sb
            cT = constp.tile([P, P], f32, tag="cT")
            nc.vector.tensor_mul(out=cT[:], in0=gate_sb[0][:], in1=gate_sb[2][:])
            tcT = constp.tile([P, P], f32, tag="tcT")
            nc.scalar.activation(out=tcT[:], in_=cT[:], func=AF.Tanh)
            wnT = constp.tile([P, P], f32, tag="wnT")
            nc.vector.tensor_mul(out=wnT[:], in0=gate_sb[3][:], in1=tcT[:])
            wn_ps = psB.tile([P, P], f32, tag="psb")
            nc.tensor.transpose(out=wn_ps[:], in_=wnT[:], identity=ident[:])
            wn_sb = constp.tile([P, P], bf16, tag="wn")
            nc.vector.tensor_copy(out=wn_sb[:], in_=wn_ps[:])

            # --- main: chunks of blocks; self pass + 2 gather sections ---
            do_pe = variant in ("full", "noscat", "deadsoh")
            do_soh = variant in ("full", "gatherdve", "deadsoh")
            use_soh = variant == "full"
            do_gather = variant != "nogather"

            # greedy min-load queue assignment: with ~8 calls/section a
            # plain n_call%nq pins every partial (light) tail call to the
            # same queue, and the other queues' extra rows set the makespan
            call_sizes = []
            for sec in sections:
                st = sec["tiles"]
                for c0 in range(0, st, CALL_T):
                    call_sizes.append(min(CALL_T, st - c0))
            qload = [0] * nq
            call_q = []
            for sz in call_sizes:
                q = min(range(nq), key=lambda k: qload[k])
                call_q.append(q)
                qload[q] += sz

            def emit_main(_iv=None):
              s_tile = 0  # global edge-tile cursor
              s_col = 0  # global one-hot column cursor
              n_call = 0  # gather call counter (queue round-robin)
              ci = 0
              b0 = 0
              for cs in chunk_sizes:
                if do_pe:
                    aggs = [
                        psA.tile([P, P], f32, name=f"agg{i}", tag="agg")
                        for i in range(cs)
                    ]
                    # self-loop pass: aggT[b] = (dinv*x_block)^T; the
                    # second dinv factor is applied at the flush
                    for i in range(cs):
                        b = b0 + i
                        xs = selfp.tile([P, P], bf16, tag="xself")
                        nc.sync.dma_start(
                            out=xs[:], in_=xself[b * P : (b + 1) * P, :]
                        )
                        nc.tensor.matmul(
                            out=aggs[i][:], lhsT=xs[:], rhs=identb[:],
                            start=True, stop=False,
                        )
                for r in range(2):
                    sec = sections[2 * ci + r]
                    sec_tiles = sec["tiles"]
                    idx_t = metap.tile([P, sec_tiles * 8], i16, tag="idx")
                    nc.sync.dma_start(
                        out=idx_t[:],
                        in_=gidx[:, s_tile * 8 : (s_tile + sec_tiles) * 8],
                    )
                    stag = stagp.tile([P, sec_tiles * P], bf16, tag="stag")
                    for c0 in range(0, sec_tiles, CALL_T):
                        if not do_gather:
                            break
                        ct = min(CALL_T, sec_tiles - c0)
                        nc.gpsimd.dma_gather(
                            out_ap=stag[:, c0 * P : (c0 + ct) * P].rearrange(
                                "p (t c) -> p t c", t=ct
                            ),
                            in_ap=xsrc[bases[r] :, :],
                            idxs_ap=idx_t[:, c0 * 8 : (c0 + ct) * 8],
                            num_idxs=ct * P,
                            num_idxs_reg=ct * P,
                            elem_size=P,
                            queue_num=call_q[n_call],
                        )
                        n_call += 1
                    if not (do_pe or do_soh):
                        s_tile += sec_tiles
                        continue
                    if do_soh:
                        # host-precomputed one-hot scatter tiles, streamed
                        # on the ACT HWDGE ring (sync ring would head-of-
                        # line-block the idx loads; on-device DVE build
                        # would starve SWDGE desc-gen via the shared port)
                        s_oh = workp.tile([P, sec["ncols"] * P], fp8, tag="soh")
                        nc.scalar.dma_start(
                            out=s_oh[:],
                            in_=gsoh[:, s_col * P : (s_col + sec["ncols"]) * P],
                        )
                    if do_pe:
                        # stop flag goes on each block's last scatter matmul
                        last_of = {}
                        if r == 1:
                            for j, cov in enumerate(sec["covers"]):
                                for i, col in cov:
                                    last_of[i] = (j, col)
                        for j, cov in enumerate(sec["covers"]):
                            for i, col in cov:
                                nc.tensor.matmul(
                                    out=aggs[i][:],
                                    lhsT=stag[:, j * P : (j + 1) * P],
                                    rhs=s_oh[:, col * P : (col + 1) * P]
                                    if use_soh
                                    else identb[:],
                                    start=False,
                                    stop=(r == 1 and last_of.get(i) == (j, col)),
                                )
                    s_tile += sec_tiles
                    s_col += sec["ncols"]
                # flush chunk
                for i in range(cs if do_pe else 0):
                    b = b0 + i
                    agg_sb = osbp.tile([P, P], bf16, tag="aggsb")
                    nc.scalar.activation(out=agg_sb[:], in_=aggs[i][:], func=AF.Copy)
                    y_ps = psB.tile([P, P], f32, tag="psb")
                    nc.tensor.matmul(
                        out=y_ps[:], lhsT=agg_sb[:], rhs=wn_sb[:],
                        start=True, stop=True,
                    )
                    y_sb = osbp.tile([P, P], f32, tag="ysb")
                    nc.scalar.activation(
                        out=y_sb[:], in_=y_ps[:], func=AF.Copy,
                        scale=d2_sb[:, b : b + 1],
                    )
                    nc.sync.dma_start(
                        out=out[b * P : (b + 1) * P, :], in_=y_sb[:]
                    )
                b0 += cs
                ci += 1

            if reps > 1:
                with tc.For_i(0, reps, 1):
                    emit_main()
            else:
                emit_main()

    nc.finalize()
    return nc


def kernel(**inputs) -> np.ndarray:
    from concourse.bass_utils import run_bass_kernel_spmd

    x = inputs["x"]
    n = x.shape[0]
    in_maps, meta = prep_inputs(
        x,
        inputs["edge_index"],
        inputs["weight"],
        inputs["w_ih"],
        inputs["b_ih"],
        inputs["b_hh"],
        n=n,
    )
    nc = build_program(meta)
    res = run_bass_kernel_spmd(nc, in_maps, list(range(N_CORES)))
    full = np.concatenate([r["out"] for r in res.results], axis=0)
    return np.ascontiguousarray(full[:n])
